# revision 39
# baseline (speedup 1.0000x reference)
"""Trainium2 Bass kernel for nn_Align_54279796687162 (sparse_attention).

Pure data parallel: one sample per NeuronCore (B=8 over 8 cores).
v3: all-bf16 datapath. cb/cf SBUF-resident; shunts computed in phase A via
linearity (shunt(xf) = shunt(cb) + attnT @ shunt(cf)); q/k/v computed
directly from cb/cf (q = wq@cb + (attnT@wq^T)^T@cf); single PE-bound
mega-loop: qkv -> DW (diag matmuls) -> PW -> xx -> proj -> out.
"""

import numpy as np
import ml_dtypes

import concourse.bass as bass
import concourse.mybir as mybir
import concourse.tile as tile
from concourse import bacc
from concourse.bass_utils import run_bass_kernel_spmd

BF = mybir.dt.bfloat16
F32 = mybir.dt.float32
AF = mybir.ActivationFunctionType
ALU = mybir.AluOpType
AX = mybir.AxisListType

H = W = 128
N = H * W            # 16384
BL = 512             # block size (4 rows * 128)
CH = 4               # chunks
SCALE = 0.25         # KD ** -0.5
PST = 132            # padded row stride for q/k/v (DW conv layout)
PSZ = PST * 130      # padded tensor size per partition
SST = 8 * PST        # q/k ring slot stride (8 rows: 4 data + 2+2 halo)

# bias column map in the packed [128, 20] f32 bias tile
B_CCAM, B_ENC, B_Q, B_K, B_V, B_DW, B_PW, B_ROW, B_COL, B_PROJ3 = (
    0, 2, 3, 4, 5, 7, 11, 13, 15, 17)

_CACHE = {}


def _ap(base, extra_off, free_dims):
    """Build an AP from a tile's base AP with custom free dims."""
    b = base[:]
    return bass.AP(b.tensor, b.offset + extra_off, [list(b.ap[0])] + free_dims)


def build_graph(scale_ccam: float):
    nc = bacc.Bacc(None, target_bir_lowering=False)

    xb = nc.dram_tensor("xb", [128, N], F32, kind="ExternalInput")
    w3t = nc.dram_tensor("w3t", [128, 9 * 256], BF, kind="ExternalInput")
    wenc = nc.dram_tensor("wenc", [128, 32], BF, kind="ExternalInput")
    wq = nc.dram_tensor("wq", [128, 256], BF, kind="ExternalInput")
    wk = nc.dram_tensor("wk", [128, 256], BF, kind="ExternalInput")
    wv = nc.dram_tensor("wv", [128, 512], BF, kind="ExternalInput")
    dwd = nc.dram_tensor("dwd", [128, 36 * 128], BF, kind="ExternalInput")
    wpw = nc.dram_tensor("wpw", [128, 4 * 256], BF, kind="ExternalInput")
    wqs = nc.dram_tensor("wqs", [128, 256], BF, kind="ExternalInput")
    wks = nc.dram_tensor("wks", [128, 256], BF, kind="ExternalInput")
    wvs = nc.dram_tensor("wvs", [128, 512], BF, kind="ExternalInput")
    wrow = nc.dram_tensor("wrow", [128, 512], BF, kind="ExternalInput")
    wcol = nc.dram_tensor("wcol", [128, 512], BF, kind="ExternalInput")
    wproj = nc.dram_tensor("wproj", [128, 512], BF, kind="ExternalInput")
    post = nc.dram_tensor("post", [16, 4 * 512], BF, kind="ExternalInput")
    interpm = nc.dram_tensor("interpm", [16, 128], BF, kind="ExternalInput")
    identb = nc.dram_tensor("identb", [128, 128], BF, kind="ExternalInput")
    identf = nc.dram_tensor("identf", [128, 128], F32, kind="ExternalInput")
    onesb = nc.dram_tensor("onesb", [128, 1], BF, kind="ExternalInput")
    biases = nc.dram_tensor("biases", [128, 20], F32, kind="ExternalInput")

    cb_dram = nc.dram_tensor("cb_dram", [2, 128, N], BF, kind="Internal")
    out = nc.dram_tensor("out", [256, N], F32, kind="ExternalOutput")

    with tile.TileContext(nc) as tc:
      with tc.tile_pool(name="cst", bufs=1) as cst:
        wenc_s = cst.tile([128, 32], BF)
        wqb_s = cst.tile([128, 256], BF)
        wkb_s = cst.tile([128, 256], BF)
        wvb_s = cst.tile([128, 512], BF)
        wproj_s = cst.tile([128, 512], BF)
        idb_s = cst.tile([128, 128], BF)
        ones_s = cst.tile([128, 1], BF)
        bia_s = cst.tile([128, 20], F32)
        for t, d in [(wenc_s, wenc), (wqb_s, wq), (wkb_s, wk), (wvb_s, wv),
                     (wproj_s, wproj), (idb_s, identb),
                     (ones_s, onesb), (bia_s, biases)]:
            nc.sync.dma_start(t[:], d[:])

        wqs_s = cst.tile([128, 256], BF)
        wks_s = cst.tile([128, 256], BF)
        wvs_s = cst.tile([128, 512], BF)
        wrow_s = cst.tile([128, 512], BF)
        wcol_s = cst.tile([128, 512], BF)
        post_s = cst.tile([16, 4 * 512], BF)
        interp_s = cst.tile([16, 128], BF)
        for t, d in [(wqs_s, wqs), (wks_s, wks), (wvs_s, wvs),
                     (wrow_s, wrow), (wcol_s, wcol), (post_s, post),
                     (interp_s, interpm)]:
            nc.sync.dma_start(t[:], d[:])
        attnT_s = cst.tile([16, 256], BF)
        aq_s = cst.tile([16, 512], BF)       # (attnT @ w{q,k,v}^T) per target
        xfs_row = [cst.tile([128, 512], BF, tag=f"xfsr{h}", name=f"xfsr{h}") for h in range(2)]
        xfs_col = [cst.tile([128, 512], BF, tag=f"xfsc{h}", name=f"xfsc{h}") for h in range(2)]
        xproj = {(d_, t_): cst.tile([128, 512], BF, tag=f"xp{d_}{t_}", name=f"xp{d_}{t_}")
                 for d_ in range(2) for t_ in range(2)}

        with tc.tile_pool(name="pmid", bufs=1) as pmid:
          cf = pmid.tile([16, N], BF)

          # =========================================================
          # Phase A: conv3x3 -> cb ; cf ; shunts of cb/cf ; energy ;
          #          ccam softmax ; xfs assembly ; AQ
          # =========================================================
          with (
              tc.tile_pool(name="pa", bufs=1) as pa,
              tc.tile_pool(name="par", bufs=3) as par,
              tc.tile_pool(name="pamm", bufs=4, space="PSUM") as pamm,
              tc.tile_pool(name="patr", bufs=2, space="PSUM") as patr,
              tc.tile_pool(name="pae", bufs=2, space="PSUM") as pae,
          ):
            cb = [pa.tile([128, N], BF, tag=f"cb{h}", name=f"cb{h}")
                  for h in range(2)]
            xpad = pa.tile([128, 130 * 130], BF)
            w3_s = pa.tile([128, 9 * 256], BF)
            idf_s = pa.tile([128, 128], F32)
            nc.sync.dma_start(w3_s[:], w3t[:])
            nc.sync.dma_start(idf_s[:], identf[:])

            # zero only the pad border of xpad; DMA x (f32->bf16) straight
            # into the interior, 32 rows at a time
            nc.vector.memset(_ap(xpad, 0, [[1, 130]]), 0.0)
            nc.vector.memset(_ap(xpad, 129 * 130, [[1, 130]]), 0.0)
            nc.vector.memset(_ap(xpad, 130, [[130, 128], [129, 2]]), 0.0)
            for rc in range(4):
                nc.gpsimd.dma_start(
                    _ap(xpad, 131 + rc * 32 * 130, [[130, 32], [1, 128]]),
                    _ap(xb, rc * 32 * 128, [[128, 32], [1, 128]]))

            scb_row = [pa.tile([128, 512], BF, tag=f"sbr{h}", name=f"sbr{h}")
                       for h in range(2)]
            scb_col = [pa.tile([128, 512], F32, tag=f"sbc{h}", name=f"sbc{h}")
                       for h in range(2)]
            scf_row = pa.tile([16, 512], BF)
            scf_col = pa.tile([16, 512], F32)
            scf_colb = pa.tile([16, 512], BF)

            # conv3x3: contiguous padded windows (junk cols stripped by
            # the ACT extraction copy), tap-major over 4-block psum groups
            cblk = [(r0, 3) for r0 in range(0, 126, 3)] + [(126, 2)]
            for half in range(2):
                for g0 in range(0, len(cblk), 3):
                    grp = cblk[g0:g0 + 3]
                    pss = [pamm.tile([128, BL], F32, tag="amm",
                                     name=f"cps{j}")
                           for j in range(len(grp))]
                    for t9 in range(9):
                        ky, kx = divmod(t9, 3)
                        for j, (r0, nr) in enumerate(grp):
                            rhs = _ap(xpad, (r0 + ky) * 130 + kx,
                                      [[1, nr * 130 - 2]])
                            nc.tensor.matmul(
                                _ap(pss[j], 0, [[1, nr * 130 - 2]]),
                                w3_s[:, t9 * 256 + half * 128:
                                     t9 * 256 + half * 128 + 128],
                                rhs, start=(t9 == 0), stop=(t9 == 8))
                    for j, (r0, nr) in enumerate(grp):
                        nc.scalar.activation(
                            cb[half][:, r0 * 128:(r0 + nr) * 128],
                            _ap(pss[j], 0, [[130, nr], [1, 128]]),
                            AF.Relu,
                            bias=bia_s[:, B_CCAM + half:B_CCAM + half + 1])
                nc.sync.dma_start(cb_dram[half, :, :], cb[half][:])

            # shunts of cb (row: mean over W%4 chunks; col: mean over H%4)
            # run on DVE/GpSimd while PE does enc + energy transposes
            for half in range(2):
                for b in range(32):
                    sl = slice(b * BL, (b + 1) * BL)
                    with nc.allow_low_precision(reason="bf16 shunt sums"):
                        src = _ap(cb[half], b * BL,
                                  [[1, 4], [128, 4], [4, 32]])
                        dst = _ap(scb_row[half], 4 * b,
                                  [[128, 4], [1, 4]])
                        nc.vector.tensor_reduce(dst, src, axis=AX.X,
                                                op=ALU.add)
                    ci = b // 8
                    part = par.tile([128, 128], F32, tag=f"cp{half}",
                                    name=f"cp{half}", bufs=2)
                    src = _ap(cb[half], b * BL, [[1, 128], [128, 4]])
                    nc.vector.tensor_reduce(part[:], src, axis=AX.X,
                                            op=ALU.add)
                    dstc = scb_col[half][:, ci * 128:(ci + 1) * 128]
                    if b % 8 == 0:
                        nc.gpsimd.tensor_copy(dstc, part[:])
                    else:
                        nc.gpsimd.tensor_tensor(dstc, dstc, part[:],
                                                ALU.add)

            # cf = relu(w_enc @ cb + b_enc)  -> [16, N]
            for bg in range(8):
                pss = [pamm.tile([16, BL], F32, tag="amm",
                                 name=f"fps{j}") for j in range(4)]
                for half in range(2):
                    for j in range(4):
                        b = bg * 4 + j
                        nc.tensor.matmul(
                            pss[j][:], wenc_s[:, half * 16:half * 16 + 16],
                            cb[half][:, b * BL:(b + 1) * BL],
                            start=(half == 0), stop=(half == 1))
                for j in range(4):
                    b = bg * 4 + j
                    nc.scalar.activation(
                        cf[:, b * BL:(b + 1) * BL], pss[j][:], AF.Relu,
                        bias=bia_s[:16, B_ENC:B_ENC + 1])

            # shunts of cf
            for b in range(32):
                with nc.allow_low_precision(reason="bf16 shunt sums"):
                    src = _ap(cf, b * BL, [[1, 4], [128, 4], [4, 32]])
                    dst = _ap(scf_row, 4 * b, [[128, 4], [1, 4]])
                    nc.vector.tensor_reduce(dst, src, axis=AX.X, op=ALU.add)
                ci = b // 8
                partf = par.tile([16, 128], F32, tag="cpf", bufs=2)
                src = _ap(cf, b * BL, [[1, 128], [128, 4]])
                nc.vector.tensor_reduce(partf[:], src, axis=AX.X, op=ALU.add)
                dstc = scf_col[:, ci * 128:(ci + 1) * 128]
                if b % 8 == 0:
                    nc.gpsimd.tensor_copy(dstc, partf[:])
                else:
                    nc.gpsimd.tensor_tensor(dstc, dstc, partf[:], ALU.add)

            # energy^T [16, 256] accumulated over 128 column-blocks.
            e_chain = [pae.tile([16, 256], F32, tag="ech", name=f"ech{c}")
                       for c in range(2)]
            for b in range(128):
                sl = slice(b * 128, (b + 1) * 128)
                tball = patr.tile([128, 272], BF, tag="tr")
                nc.tensor.matmul(tball[:, 0:128], cb[0][:, sl], idb_s[:],
                                 is_transpose=True, start=True, stop=False)
                nc.tensor.matmul(tball[:, 128:256], cb[1][:, sl], idb_s[:],
                                 is_transpose=True, start=False, stop=False)
                nc.tensor.matmul(tball[:, 256:272], cf[:, sl],
                                 idb_s[:16, :16],
                                 is_transpose=True, start=False, stop=True)
                bT = par.tile([128, 272], BF, tag="bT")
                nc.scalar.activation(bT[:], tball[:], AF.Copy)
                nc.tensor.matmul(e_chain[b % 2][:], bT[:, 256:272],
                                 bT[:, 0:256],
                                 start=(b < 2), stop=(b >= 126))

            # CCAM attention: attn = softmax(-energy) over K=16, store attn^T
            e_sb = pa.tile([16, 256], F32)
            e_tmp = pa.tile([16, 256], F32)
            nc.scalar.activation(e_tmp[:], e_chain[1][:], AF.Copy)
            nc.vector.tensor_tensor(e_sb[:], e_chain[0][:], e_tmp[:], ALU.add)
            at_half = []
            hs_ = {}
            for half in range(2):
                tps = patr.tile([128, 16], F32, tag="tr", name=f"tps{half}")
                nc.tensor.transpose(
                    tps[:], e_sb[:, half * 128:(half + 1) * 128],
                    idf_s[:16, :16])
                hs_[half] = dict(tps=tps)
            for half in range(2):
                e_c = par.tile([128, 16], F32, tag="ec", name=f"ec{half}")
                nc.vector.tensor_copy(e_c[:], hs_[half]["tps"][:])
                hs_[half]["ec"] = e_c
            for half in range(2):
                mn = par.tile([128, 1], F32, tag="mn", name=f"mn{half}")
                nc.vector.tensor_reduce(mn[:], hs_[half]["ec"][:],
                                        axis=AX.X, op=ALU.min)
                hs_[half]["mn"] = mn
            for half in range(2):
                ex = par.tile([128, 16], F32, tag="ex", name=f"ex{half}")
                nc.scalar.activation(ex[:], hs_[half]["ec"][:], AF.Exp,
                                     bias=hs_[half]["mn"][:], scale=-1.0)
                hs_[half]["ex"] = ex
            for half in range(2):
                sm = par.tile([128, 1], F32, tag="sm", name=f"sm{half}")
                nc.vector.tensor_reduce(sm[:], hs_[half]["ex"][:],
                                        axis=AX.X, op=ALU.add)
                rc = par.tile([128, 1], F32, tag="rc", name=f"rc{half}")
                nc.vector.reciprocal(rc[:], sm[:])
                hs_[half]["rc"] = rc
            for half in range(2):
                at = par.tile([128, 16], BF, tag="at", bufs=2,
                              name=f"at{half}")
                nc.vector.tensor_scalar(at[:], hs_[half]["ex"][:],
                                        hs_[half]["rc"][:],
                                        float(scale_ccam), ALU.mult, ALU.mult)
                at_half.append(at)
            for half in range(2):
                tat = patr.tile([16, 128], BF, tag="tr", name=f"tat{half}")
                nc.tensor.transpose(tat[:], at_half[half][:], idb_s[:])
                nc.vector.tensor_copy(
                    attnT_s[:, half * 128:(half + 1) * 128], tat[:])

            # AQ[m] = attnT @ w_m^T  -> [16, 4*128] (targets q,k,v0,v1)
            aq_ps = pae.tile([16, 512], F32, tag="ech", name="aq_ps")
            for m, (wt, mt) in enumerate([(wqb_s, 0), (wkb_s, 0),
                                          (wvb_s, 0), (wvb_s, 1)]):
                nt = 2 if wt is wvb_s else 1
                for kh in range(2):
                    nc.tensor.matmul(
                        aq_ps[:, m * 128:(m + 1) * 128], at_half[kh][:],
                        wt[:, (kh * nt + mt) * 128:(kh * nt + mt) * 128 + 128],
                        start=(kh == 0), stop=(kh == 1))
            nc.scalar.activation(aq_s[:], aq_ps[:], AF.Copy)

            # xfs = shunt(cb) + attnT @ shunt(cf)
            nc.vector.tensor_copy(scf_colb[:], scf_col[:])
            for half in range(2):
                ps = pamm.tile([128, BL], F32, tag="amm", name="xfr")
                nc.tensor.matmul(ps[:],
                                 attnT_s[:, half * 128:(half + 1) * 128],
                                 scf_row[:], start=True, stop=False)
                nc.tensor.matmul(ps[:], idb_s[:], scb_row[half][:],
                                 start=False, stop=True)
                nc.scalar.activation(xfs_row[half][:], ps[:], AF.Copy)
                ps2 = pamm.tile([128, BL], F32, tag="amm", name="xfc")
                nc.tensor.matmul(ps2[:],
                                 attnT_s[:, half * 128:(half + 1) * 128],
                                 scf_colb[:], start=True, stop=True)
                with nc.allow_low_precision(reason="bf16 xfs_col"):
                    nc.vector.tensor_tensor(xfs_col[half][:], ps2[:],
                                            scb_col[half][:], ALU.add)

          # =========================================================
          # Phase C1: axial attention (row: dir 0, col: dir 1)
          # =========================================================
          with (
              tc.tile_pool(name="pc", bufs=1) as pc,
              tc.tile_pool(name="pcr", bufs=3) as pcr,
              tc.tile_pool(name="pcm", bufs=2, space="PSUM") as pcm,
          ):
            pcmm = pcl = pcav = pcasm = pcm
            # --- interleave the two independent axial directions so one
            # direction's matmuls fill the other's pipeline latency ---
            st = {}
            for d_ in range(2):
                xfs = xfs_row if d_ == 0 else xfs_col
                qs_att = pc.tile([128, 512], BF, tag="qsa", name="qsa",
                                 bufs=2)
                ks_att = pc.tile([128, 512], BF, tag="ksa", name="ksa",
                                 bufs=2)
                vs_att = [pc.tile([128, 512], BF, tag=f"vsa{h}",
                                  name=f"vsa{h}", bufs=2)
                          for h in range(2)]
                st[d_] = dict(xfs=xfs, qs=qs_att, ks=ks_att, vs=vs_att)
                for (dst, wt, bc, nt, pidx) in [
                        ([qs_att], wqs_s, B_Q, 1, 2 * d_),
                        ([ks_att], wks_s, B_K, 1, 2 * d_ + 1),
                        (vs_att, wvs_s, B_V, 2, None)]:
                    for mt in range(nt):
                        ps = pcmm.tile([128, BL], F32, tag="cmm", bufs=2)
                        for kh in range(2):
                            nc.tensor.matmul(
                                ps[:],
                                wt[:, (kh * nt + mt) * 128:
                                   (kh * nt + mt) * 128 + 128],
                                xfs[kh][:], start=(kh == 0),
                                stop=(kh == 1 and pidx is None))
                        if pidx is not None:
                            for i in range(CH):
                                nc.tensor.matmul(
                                    ps[:, i * 128:(i + 1) * 128],
                                    post_s[:, (pidx * 4 + i) * 128:
                                           (pidx * 4 + i) * 128 + 128],
                                    interp_s[:], start=False, stop=(i == 3))
                        nc.scalar.activation(
                            dst[mt][:], ps[:], AF.Identity,
                            bias=bia_s[:, bc + mt:bc + mt + 1])

            for d_ in range(2):
                q_pack = pc.tile([128, 1024], BF, tag="qp", name="qp",
                                 bufs=2)
                k_pack = pc.tile([128, 1024], BF, tag="kp", name="kp",
                                 bufs=2)
                st[d_]["qp"] = q_pack
                st[d_]["kp"] = k_pack
                for g in range(8):
                    po, co = 32 * (g % 4), (g // 4) * 512
                    nc.sync.dma_start(
                        q_pack[po:po + 16, co:co + 512],
                        st[d_]["qs"][g * 16:(g + 1) * 16, :])
                    nc.sync.dma_start(
                        k_pack[po:po + 16, co:co + 512],
                        st[d_]["ks"][g * 16:(g + 1) * 16, :])

            for d_ in range(2):
                vt_s = pc.tile([128, 4, 256], BF, tag="vt", name="vt",
                               bufs=2)
                st[d_]["vt"] = vt_s
                for i in range(CH):
                    for hh in range(2):
                        tp = pcl.tile([128, 128], BF, tag="lps")
                        nc.tensor.transpose(
                            tp[:], st[d_]["vs"][hh][:, i * 128:(i + 1) * 128],
                            idb_s[:])
                        nc.scalar.activation(
                            vt_s[:, i, hh * 128:(hh + 1) * 128], tp[:],
                            AF.Copy)
                st[d_]["xpre"] = [
                    pc.tile([128, 512], BF, tag=f"xpre{t}",
                            name=f"xpre{t}", bufs=2) for t in range(2)]

            for i in range(CH):
                for th in range(2):
                    asm_ps = {d_: pcasm.tile([128, 128], BF, tag="asm",
                                             name=f"asm{d_}", bufs=2)
                              for d_ in range(2)}
                    for gg in range(4):
                        g = th * 4 + gg
                        po = 32 * (g % 4)
                        co = (g // 4) * 512
                        sl_gi = slice(co + i * 128, co + i * 128 + 128)
                        lps = {}
                        ets = {}
                        avs = {}
                        for d_ in range(2):
                            l_ps = pcl.tile([128, 128], F32, tag="lps",
                                            name=f"lps{d_}")
                            nc.tensor.matmul(l_ps[:],
                                             st[d_]["kp"][po:po + 16, sl_gi],
                                             st[d_]["qp"][po:po + 16, sl_gi],
                                             start=True, stop=True,
                                             tile_position=(po, 0))
                            lps[d_] = l_ps
                        for d_ in range(2):
                            e_t = pcr.tile([128, 128], BF, tag="et",
                                           name=f"et{d_}", bufs=4)
                            nc.scalar.activation(e_t[:], lps[d_][:], AF.Exp,
                                                 scale=SCALE)
                            ets[d_] = e_t
                        for d_ in range(2):
                            av_ps = pcav.tile([128, 33], F32, tag="av",
                                              name=f"av{d_}")
                            nc.tensor.matmul(
                                av_ps[:, 0:32], ets[d_][:],
                                st[d_]["vt"][:, i, g * 32:(g + 1) * 32],
                                start=True, stop=False)
                            nc.tensor.matmul(av_ps[:, 32:33], ets[d_][:],
                                             ones_s[:], start=False,
                                             stop=True)
                            avs[d_] = av_ps
                        xrns = {}
                        for d_ in range(2):
                            rcp = pcr.tile([128, 1], F32, tag="rcp",
                                           name=f"rcp{d_}", bufs=4)
                            nc.vector.reciprocal(rcp[:], avs[d_][:, 32:33])
                            xrn = pcr.tile([128, 32], BF, tag="xrn",
                                           name=f"xrn{d_}", bufs=4)
                            nc.vector.tensor_scalar(
                                xrn[:], avs[d_][:, 0:32], rcp[:], None,
                                ALU.mult)
                            xrns[d_] = xrn
                        for d_ in range(2):
                            nc.tensor.transpose(
                                asm_ps[d_][gg * 32:(gg + 1) * 32, :],
                                xrns[d_][:], idb_s[:],
                                tile_position=(0, gg * 32))
                    for d_ in range(2):
                        nc.scalar.activation(
                            st[d_]["xpre"][th][:, i * 128:(i + 1) * 128],
                            asm_ps[d_][:], AF.Relu)

            for d_ in range(2):
                wproj_d = wrow_s if d_ == 0 else wcol_s
                bcol = B_ROW if d_ == 0 else B_COL
                for mt in range(2):
                    ps = pcmm.tile([128, BL], F32, tag="cmm", bufs=2)
                    for kh in range(2):
                        nc.tensor.matmul(
                            ps[:],
                            wproj_d[:, (kh * 2 + mt) * 128:
                                    (kh * 2 + mt) * 128 + 128],
                            st[d_]["xpre"][kh][:], start=(kh == 0),
                            stop=(kh == 1))
                    nc.scalar.activation(
                        xproj[(d_, mt)][:], ps[:], AF.Identity,
                        bias=bia_s[:, bcol + mt:bcol + mt + 1])

          # =========================================================
          # Mega-loop: qkv[b] -> DW[b-1] -> PW[b-1] -> xx -> proj -> out
          # =========================================================
          with (
              tc.tile_pool(name="pf", bufs=1) as pf,
              tc.tile_pool(name="pfr", bufs=3) as pfr,
              tc.tile_pool(name="pbm", bufs=4, space="PSUM") as pbm,
              tc.tile_pool(name="pfd", bufs=3, space="PSUM") as pfd,
              tc.tile_pool(name="pfw", bufs=2, space="PSUM") as pfw,
          ):
            dwd_s = pf.tile([128, 36 * 128], BF)
            wpw_s = pf.tile([128, 4 * 256], BF)
            nc.sync.dma_start(dwd_s[:], dwd[:])
            nc.sync.dma_start(wpw_s[:], wpw[:])
            v_sb = [pf.tile([128, PSZ], BF, tag=f"v{h}", name=f"v{h}")
                    for h in range(2)]
            for t_ in v_sb:
                nc.gpsimd.memset(_ap(t_, 0, [[129 * PST, 2], [1, PST]]), 0.0)
                nc.gpsimd.memset(
                    _ap(t_, PST, [[PST, 128], [130, 2], [1, 2]]), 0.0)
            # q/k rings: 3 slots of 8 rows (2 halo + 4 data + 2 halo)
            qring = [pf.tile([128, SST], BF, name=f"qr{i}") for i in range(3)]
            kring = [pf.tile([128, SST], BF, name=f"kr{i}") for i in range(3)]
            for t_ in qring + kring:
                nc.gpsimd.memset(t_[:], 0.0)

            cbq = []
            def fetch_cb(bb):
                sl2 = slice(bb * BL, (bb + 1) * BL)
                t2 = [pfr.tile([128, BL], BF, tag=f"cbi{h}",
                               name=f"cbi{h}", bufs=4) for h in range(2)]
                nc.sync.dma_start(t2[0][:], cb_dram[0, :, sl2])
                nc.sync.dma_start(t2[1][:], cb_dram[1, :, sl2])
                cbq.append(t2)
            fetch_cb(0)
            fetch_cb(1)
            fetch_cb(2)
            for b in range(33):
              if b < 32:
                sl = slice(b * BL, (b + 1) * BL)
                if b + 3 < 32:
                    fetch_cb(b + 3)
                cbi = cbq[b]
                # q/k/v block b: 2 matmuls K=128 over cb halves + 1 K=16 (cf)
                for m, (wt, mt) in enumerate([(wqb_s, 0), (wkb_s, 0),
                                              (wvb_s, 0), (wvb_s, 1)]):
                    nt = 2 if wt is wvb_s else 1
                    ps = pbm.tile([128, BL], F32, tag="bmm",
                                  name=f"qps{m}")
                    for kh in range(2):
                        nc.tensor.matmul(
                            ps[:],
                            wt[:, (kh * nt + mt) * 128:
                               (kh * nt + mt) * 128 + 128],
                            cbi[kh][:], start=(kh == 0), stop=False)
                    nc.tensor.matmul(ps[:], aq_s[:, m * 128:(m + 1) * 128],
                                     cf[:, sl], start=False, stop=True)
                    bc = (B_Q, B_K, B_V, B_V + 1)[m]
                    if m < 2:
                        ring = qring if m == 0 else kring
                        pdst = _ap(ring[b % 3], 2 * PST + 2,
                                   [[PST, 4], [1, 128]])
                    else:
                        pdst = _ap(v_sb[m - 2], (4 * b + 1) * PST + 2,
                                   [[PST, 4], [1, 128]])
                    if m % 2 == 0:
                        nc.scalar.activation(
                            pdst, ps[:], AF.Identity,
                            bias=bia_s[:, bc:bc + 1])
                    else:
                        nc.vector.tensor_scalar(
                            pdst, ps[:], bia_s[:, bc:bc + 1],
                            None, ALU.add)
                # halo copies: slot b rows 0..1 <- slot b-1 rows 4..5;
                #              slot b-1 rows 6..7 <- slot b rows 2..3
                for ring in (qring, kring):
                    if b > 0:
                        nc.gpsimd.tensor_copy(
                            _ap(ring[b % 3], 0, [[1, 2 * PST]]),
                            _ap(ring[(b - 1) % 3], 4 * PST, [[1, 2 * PST]]))
                        nc.gpsimd.tensor_copy(
                            _ap(ring[(b - 1) % 3], 6 * PST, [[1, 2 * PST]]),
                            _ap(ring[b % 3], 2 * PST, [[1, 2 * PST]]))
                    else:
                        nc.gpsimd.memset(
                            _ap(ring[0], 0, [[1, 2 * PST]]), 0.0)
              if b == 0:
                  continue
              bp = b - 1          # tail block
              if bp == 31:
                  for ring in (qring, kring):
                      nc.gpsimd.memset(
                          _ap(ring[bp % 3], 6 * PST, [[1, 2 * PST]]), 0.0)

              # --- DW for block bp: 2-row groups j=0,1 ---
              dwg = [pfr.tile([128, 4 * 256], BF, tag=f"dwg{j}",
                              name=f"dwg{j}", bufs=2) for j in range(2)]
              for t in range(4):
                  dps = [pfd.tile([128, 260], F32, tag="dw",
                                  name=f"dps{j}", bufs=2) for j in range(2)]
                  for t9 in range(9):
                      ky, kx = divmod(t9, 3)
                      for j in range(2):
                          if t < 2:
                              src = (qring, kring)[t][bp % 3]
                              off = (1 + 2 * j + ky) * PST + kx + 1
                          else:
                              src = v_sb[t - 2]
                              off = (4 * bp + 2 * j + ky) * PST + kx + 1
                          nc.tensor.matmul(
                              _ap(dps[j], 0, [[1, 260]]),
                              dwd_s[:, (t * 9 + t9) * 128:
                                    (t * 9 + t9) * 128 + 128],
                              _ap(src, off, [[1, 260]]),
                              start=(t9 == 0), stop=(t9 == 8))
                  for j in range(2):
                      src = _ap(dps[j], 0, [[PST, 2], [1, 128]])
                      dst = _ap(dwg[j], t * 256, [[1, 256]])
                      if t % 2 == 0:
                          nc.scalar.activation(
                              dst, src, AF.Relu,
                              bias=bia_s[:, B_DW + t:B_DW + t + 1])
                      else:
                          nc.vector.tensor_scalar(
                              dst, src, bia_s[:, B_DW + t:B_DW + t + 1],
                              0.0, ALU.add, ALU.max)

              # --- PW: qo = w_pw @ dwg + b_pw ---
              pws = [pfw.tile([128, 2, 256], F32, tag="pw",
                              name=f"pws{j}") for j in range(2)]
              for mt in range(2):
                  for kt in range(4):
                      for j in range(2):
                          nc.tensor.matmul(
                              pws[j][:, mt, :],
                              wpw_s[:, kt * 256 + mt * 128:
                                    kt * 256 + mt * 128 + 128],
                              dwg[j][:, kt * 256:(kt + 1) * 256],
                              start=(kt == 0), stop=(kt == 3))
              qo_blk = [pfr.tile([128, BL], BF, tag=f"qo{mt}",
                                 name=f"qo{mt}", bufs=2) for mt in range(2)]
              for mt in range(2):
                  for j in range(2):
                      nc.scalar.activation(
                          qo_blk[mt][:, j * 256:(j + 1) * 256],
                          pws[j][:, mt, :], AF.Identity,
                          bias=bia_s[:, B_PW + mt:B_PW + mt + 1])

              # --- xx = relu(v + bcast(xrow) + bcast(xcol)) ---
              xxr = []
              for half in range(2):
                  xx = pfr.tile([128, BL], BF, tag=f"xx{half}",
                                name=f"xx{half}", bufs=2)
                  rap = _ap(xproj[(0, half)], bp * 16, [[1, 16], [0, 32]])
                  cap = _ap(xproj[(1, half)], (bp // 2) * 32,
                            [[0, 4], [0, 4], [1, 32]])
                  nc.vector.tensor_tensor(xx[:], rap, cap, ALU.add)
                  vap = _ap(v_sb[half], (4 * bp + 1) * PST + 2,
                            [[PST, 4], [1, 128]])
                  nc.vector.tensor_tensor(xx[:], xx[:], vap, ALU.add)
                  nc.vector.tensor_scalar(xx[:], xx[:], 0.0, None,
                                          ALU.max)
                  xxr.append(xx)

              # --- proj + hsig + multiply qo, DMA out ---
              for mt in range(2):
                  ps = pbm.tile([128, BL], F32, tag="bmm", name="prj")
                  for kh in range(2):
                      nc.tensor.matmul(
                          ps[:],
                          wproj_s[:, (kh * 2 + mt) * 128:
                                  (kh * 2 + mt) * 128 + 128],
                          xxr[kh][:], start=(kh == 0), stop=(kh == 1))
                  hs = pfr.tile([128, BL], BF, tag="hs", bufs=2)
                  nc.scalar.activation(
                      hs[:], ps[:], AF.Relu,
                      bias=bia_s[:, B_PROJ3 + mt:B_PROJ3 + mt + 1])
                  att_t = pfr.tile([128, BL], BF, tag="att", bufs=2)
                  nc.vector.tensor_scalar(att_t[:], hs[:], 6.0,
                                          1.0 / 6.0, ALU.min, ALU.mult)
                  ob = pfr.tile([128, BL], BF, tag="ob", bufs=2)
                  nc.vector.tensor_tensor(ob[:], att_t[:], qo_blk[mt][:],
                                          ALU.mult)
                  slp = slice(bp * BL, (bp + 1) * BL)
                  nc.gpsimd.dma_start(out[mt * 128:(mt + 1) * 128, slp],
                                      ob[:])

    nc.compile()
    return nc


def _interp_matrix():
    s, n = 16, 128
    src = np.clip((np.arange(n) + 0.5) * (s / n) - 0.5, 0.0, s - 1.0)
    i0 = np.floor(src).astype(np.int64)
    i1 = np.minimum(i0 + 1, s - 1)
    w = src - i0
    M = np.zeros((s, n), np.float64)
    np.add.at(M, (i0, np.arange(n)), 1.0 - w)
    np.add.at(M, (i1, np.arange(n)), w)
    return M


def _bf(x):
    return np.ascontiguousarray(np.asarray(x, np.float32).astype(
        ml_dtypes.bfloat16))


def prep_consts(inputs):
    """Host-side layout prep of all weight tensors (shared across cores)."""
    f = {k: np.asarray(v, np.float32) for k, v in inputs.items()}

    w3 = f["w_ccam_b"]                      # [256, 128, 3, 3]
    w3t = np.zeros((128, 9 * 256), np.float32)
    for ky in range(3):
        for kx in range(3):
            t9 = ky * 3 + kx
            w3t[:, t9 * 256:(t9 + 1) * 256] = w3[:, :, ky, kx].T
    wenc = np.zeros((128, 32), np.float32)  # w_enc [16, 256]
    for half in range(2):
        wenc[:, half * 16:(half + 1) * 16] = \
            f["w_enc"][:, half * 128:(half + 1) * 128].T

    def pack_lhsT(wm, nt):
        # wm [out, in]; returns [128, 2*nt*128]: [ci, (kh*nt+mt)*128+co]
        r = np.zeros((128, 2 * nt * 128), np.float32)
        for kh in range(2):
            for mt in range(nt):
                r[:, (kh * nt + mt) * 128:(kh * nt + mt) * 128 + 128] = \
                    wm[mt * 128:(mt + 1) * 128,
                       kh * 128:(kh + 1) * 128].T
        return r

    wq_p = pack_lhsT(f["w_q"], 1)
    wk_p = pack_lhsT(f["w_k"], 1)
    wv_p = pack_lhsT(f["w_v"], 2)
    wrow_p = pack_lhsT(f["w_row"], 2)
    wcol_p = pack_lhsT(f["w_col"], 2)
    wproj_p = pack_lhsT(f["w_proj"], 2)

    wpw_p = np.zeros((128, 4 * 256), np.float32)   # w_pw [256, 512]
    for kt in range(4):
        for mt in range(2):
            wpw_p[:, kt * 256 + mt * 128:kt * 256 + mt * 128 + 128] = \
                f["w_pw"][mt * 128:(mt + 1) * 128,
                          kt * 128:(kt + 1) * 128].T

    dwdg = np.zeros((128, 36 * 128), np.float32)   # w_dw [512,1,3,3]
    ii = np.arange(128)
    for t in range(4):
        for tap9 in range(9):
            ky, kx = divmod(tap9, 3)
            dwdg[ii, (t * 9 + tap9) * 128 + ii] = \
                f["w_dw"][t * 128 + ii, 0, ky, kx]

    post_p = np.zeros((16, 4 * 512), np.float32)
    for pidx, nm in enumerate(["pos_rowq", "pos_rowk", "pos_colq", "pos_colk"]):
        p = f[nm]                                   # [4, 128, 16]
        for i in range(4):
            post_p[:, (pidx * 4 + i) * 128:(pidx * 4 + i) * 128 + 128] = \
                p[i].T                              # [16, 128]

    biases = np.zeros((128, 20), np.float32)
    biases[:, B_CCAM + 0] = f["b_ccam_b"][:128]
    biases[:, B_CCAM + 1] = f["b_ccam_b"][128:]
    biases[:16, B_ENC] = f["b_enc"]
    biases[:, B_Q] = f["b_q"]
    biases[:, B_K] = f["b_k"]
    biases[:, B_V + 0] = f["b_v"][:128]
    biases[:, B_V + 1] = f["b_v"][128:]
    for t in range(4):
        biases[:, B_DW + t] = f["b_dw"][t * 128:(t + 1) * 128]
    biases[:, B_PW + 0] = f["b_pw"][:128]
    biases[:, B_PW + 1] = f["b_pw"][128:]
    biases[:, B_ROW + 0] = f["b_row"][:128]
    biases[:, B_ROW + 1] = f["b_row"][128:]
    biases[:, B_COL + 0] = f["b_col"][:128]
    biases[:, B_COL + 1] = f["b_col"][128:]
    biases[:, B_PROJ3 + 0] = f["b_proj"][:128] + 3.0
    biases[:, B_PROJ3 + 1] = f["b_proj"][128:] + 3.0

    return {
        "w3t": _bf(w3t), "wenc": _bf(wenc),
        "wq": _bf(wq_p), "wk": _bf(wk_p), "wv": _bf(wv_p),
        "wqs": _bf(wq_p / 32.0), "wks": _bf(wk_p / 32.0),
        "wvs": _bf(wv_p / 32.0),
        "dwd": _bf(dwdg), "wpw": _bf(wpw_p),
        "wrow": _bf(wrow_p), "wcol": _bf(wcol_p), "wproj": _bf(wproj_p),
        "post": _bf(post_p), "interpm": _bf(_interp_matrix()),
        "identb": _bf(np.eye(128)),
        "identf": np.eye(128, dtype=np.float32),
        "onesb": _bf(np.ones((128, 1))),
        "biases": np.ascontiguousarray(biases),
    }


def kernel(**inputs) -> np.ndarray:
    x = np.asarray(inputs["x"], np.float32)          # [8, 128, 128, 128]
    scale = float(np.asarray(inputs["scale_ccam"]).reshape(-1)[0])

    key = round(scale, 9)
    if key not in _CACHE:
        _CACHE[key] = build_graph(scale)
    nc = _CACHE[key]

    consts = prep_consts(inputs)
    in_maps = []
    for core in range(8):
        m = dict(consts)
        m["xb"] = np.ascontiguousarray(x[core].reshape(128, N))
        in_maps.append(m)

    res = run_bass_kernel_spmd(nc, in_maps, core_ids=list(range(8)))
    outs = [res.results[i]["out"].reshape(256, 128, 128) for i in range(8)]
    return np.stack(outs).astype(np.float32)


if __name__ == "__main__":
    rng = np.random.default_rng(0)
    demo = {"x": rng.standard_norm_((8, 128, 128, 128))} if False else None
    print("kernel module OK")


# revision 40
# speedup vs baseline: 1.0041x; 1.0041x over previous
"""Trainium2 Bass kernel for nn_Align_54279796687162 (sparse_attention).

Pure data parallel: one sample per NeuronCore (B=8 over 8 cores).
v3: all-bf16 datapath. cb/cf SBUF-resident; shunts computed in phase A via
linearity (shunt(xf) = shunt(cb) + attnT @ shunt(cf)); q/k/v computed
directly from cb/cf (q = wq@cb + (attnT@wq^T)^T@cf); single PE-bound
mega-loop: qkv -> DW (diag matmuls) -> PW -> xx -> proj -> out.
"""

import numpy as np
import ml_dtypes

import concourse.bass as bass
import concourse.mybir as mybir
import concourse.tile as tile
from concourse import bacc
from concourse.bass_utils import run_bass_kernel_spmd

BF = mybir.dt.bfloat16
F32 = mybir.dt.float32
AF = mybir.ActivationFunctionType
ALU = mybir.AluOpType
AX = mybir.AxisListType

H = W = 128
N = H * W            # 16384
BL = 512             # block size (4 rows * 128)
CH = 4               # chunks
SCALE = 0.25         # KD ** -0.5
PST = 132            # padded row stride for q/k/v (DW conv layout)
PSZ = PST * 130      # padded tensor size per partition
SST = 8 * PST        # q/k ring slot stride (8 rows: 4 data + 2+2 halo)

# bias column map in the packed [128, 20] f32 bias tile
B_CCAM, B_ENC, B_Q, B_K, B_V, B_DW, B_PW, B_ROW, B_COL, B_PROJ3 = (
    0, 2, 3, 4, 5, 7, 11, 13, 15, 17)

_CACHE = {}


def _ap(base, extra_off, free_dims):
    """Build an AP from a tile's base AP with custom free dims."""
    b = base[:]
    return bass.AP(b.tensor, b.offset + extra_off, [list(b.ap[0])] + free_dims)


def build_graph(scale_ccam: float):
    nc = bacc.Bacc(None, target_bir_lowering=False)

    xb = nc.dram_tensor("xb", [128, N], F32, kind="ExternalInput")
    w3t = nc.dram_tensor("w3t", [128, 9 * 256], BF, kind="ExternalInput")
    wenc = nc.dram_tensor("wenc", [128, 32], BF, kind="ExternalInput")
    wq = nc.dram_tensor("wq", [128, 256], BF, kind="ExternalInput")
    wk = nc.dram_tensor("wk", [128, 256], BF, kind="ExternalInput")
    wv = nc.dram_tensor("wv", [128, 512], BF, kind="ExternalInput")
    dwd = nc.dram_tensor("dwd", [128, 36 * 128], BF, kind="ExternalInput")
    wpw = nc.dram_tensor("wpw", [128, 4 * 256], BF, kind="ExternalInput")
    wqs = nc.dram_tensor("wqs", [128, 256], BF, kind="ExternalInput")
    wks = nc.dram_tensor("wks", [128, 256], BF, kind="ExternalInput")
    wvs = nc.dram_tensor("wvs", [128, 512], BF, kind="ExternalInput")
    wrow = nc.dram_tensor("wrow", [128, 512], BF, kind="ExternalInput")
    wcol = nc.dram_tensor("wcol", [128, 512], BF, kind="ExternalInput")
    wproj = nc.dram_tensor("wproj", [128, 512], BF, kind="ExternalInput")
    post = nc.dram_tensor("post", [16, 4 * 512], BF, kind="ExternalInput")
    interpm = nc.dram_tensor("interpm", [16, 128], BF, kind="ExternalInput")
    identb = nc.dram_tensor("identb", [128, 128], BF, kind="ExternalInput")
    identf = nc.dram_tensor("identf", [128, 128], F32, kind="ExternalInput")
    onesb = nc.dram_tensor("onesb", [128, 1], BF, kind="ExternalInput")
    biases = nc.dram_tensor("biases", [128, 20], F32, kind="ExternalInput")

    cb_dram = nc.dram_tensor("cb_dram", [2, 128, N], BF, kind="Internal")
    out = nc.dram_tensor("out", [256, N], F32, kind="ExternalOutput")

    with tile.TileContext(nc) as tc:
      with tc.tile_pool(name="cst", bufs=1) as cst:
        wenc_s = cst.tile([128, 32], BF)
        wqb_s = cst.tile([128, 256], BF)
        wkb_s = cst.tile([128, 256], BF)
        wvb_s = cst.tile([128, 512], BF)
        wproj_s = cst.tile([128, 512], BF)
        idb_s = cst.tile([128, 128], BF)
        ones_s = cst.tile([128, 1], BF)
        bia_s = cst.tile([128, 20], F32)
        for t, d in [(wenc_s, wenc), (wqb_s, wq), (wkb_s, wk), (wvb_s, wv),
                     (wproj_s, wproj), (idb_s, identb),
                     (ones_s, onesb), (bia_s, biases)]:
            nc.sync.dma_start(t[:], d[:])

        wqs_s = cst.tile([128, 256], BF)
        wks_s = cst.tile([128, 256], BF)
        wvs_s = cst.tile([128, 512], BF)
        wrow_s = cst.tile([128, 512], BF)
        wcol_s = cst.tile([128, 512], BF)
        post_s = cst.tile([16, 4 * 512], BF)
        interp_s = cst.tile([16, 128], BF)
        for t, d in [(wqs_s, wqs), (wks_s, wks), (wvs_s, wvs),
                     (wrow_s, wrow), (wcol_s, wcol), (post_s, post),
                     (interp_s, interpm)]:
            nc.sync.dma_start(t[:], d[:])
        attnT_s = cst.tile([16, 256], BF)
        aq_s = cst.tile([16, 512], BF)       # (attnT @ w{q,k,v}^T) per target
        xfs_row = [cst.tile([128, 512], BF, tag=f"xfsr{h}", name=f"xfsr{h}") for h in range(2)]
        xfs_col = [cst.tile([128, 512], BF, tag=f"xfsc{h}", name=f"xfsc{h}") for h in range(2)]
        xproj = {(d_, t_): cst.tile([128, 512], BF, tag=f"xp{d_}{t_}", name=f"xp{d_}{t_}")
                 for d_ in range(2) for t_ in range(2)}

        with tc.tile_pool(name="pmid", bufs=1) as pmid:
          cf = pmid.tile([16, N], BF)

          # =========================================================
          # Phase A: conv3x3 -> cb ; cf ; shunts of cb/cf ; energy ;
          #          ccam softmax ; xfs assembly ; AQ
          # =========================================================
          with (
              tc.tile_pool(name="pa", bufs=1) as pa,
              tc.tile_pool(name="par", bufs=3) as par,
              tc.tile_pool(name="pamm", bufs=4, space="PSUM") as pamm,
              tc.tile_pool(name="patr", bufs=2, space="PSUM") as patr,
              tc.tile_pool(name="pae", bufs=2, space="PSUM") as pae,
          ):
            cb = [pa.tile([128, N], BF, tag=f"cb{h}", name=f"cb{h}")
                  for h in range(2)]
            xpad = pa.tile([128, 130 * 130], BF)
            w3_s = pa.tile([128, 9 * 256], BF)
            idf_s = pa.tile([128, 128], F32)
            nc.sync.dma_start(w3_s[:], w3t[:])
            nc.sync.dma_start(idf_s[:], identf[:])

            # zero only the pad border of xpad; DMA x (f32->bf16) straight
            # into the interior, 32 rows at a time
            nc.vector.memset(_ap(xpad, 0, [[1, 130]]), 0.0)
            nc.vector.memset(_ap(xpad, 129 * 130, [[1, 130]]), 0.0)
            nc.vector.memset(_ap(xpad, 130, [[130, 128], [129, 2]]), 0.0)
            for rc in range(4):
                nc.gpsimd.dma_start(
                    _ap(xpad, 131 + rc * 32 * 130, [[130, 32], [1, 128]]),
                    _ap(xb, rc * 32 * 128, [[128, 32], [1, 128]]))

            scb_row = [pa.tile([128, 512], BF, tag=f"sbr{h}", name=f"sbr{h}")
                       for h in range(2)]
            scb_col = [pa.tile([128, 512], F32, tag=f"sbc{h}", name=f"sbc{h}")
                       for h in range(2)]
            scf_row = pa.tile([16, 512], BF)
            scf_col = pa.tile([16, 512], F32)
            scf_colb = pa.tile([16, 512], BF)

            # conv3x3: contiguous padded windows (junk cols stripped by
            # the ACT extraction copy), tap-major over 4-block psum groups
            cblk = [(r0, 3) for r0 in range(0, 126, 3)] + [(126, 2)]
            for half in range(2):
                for g0 in range(0, len(cblk), 3):
                    grp = cblk[g0:g0 + 3]
                    pss = [pamm.tile([128, BL], F32, tag="amm",
                                     name=f"cps{j}")
                           for j in range(len(grp))]
                    for t9 in range(9):
                        ky, kx = divmod(t9, 3)
                        for j, (r0, nr) in enumerate(grp):
                            rhs = _ap(xpad, (r0 + ky) * 130 + kx,
                                      [[1, nr * 130 - 2]])
                            nc.tensor.matmul(
                                _ap(pss[j], 0, [[1, nr * 130 - 2]]),
                                w3_s[:, t9 * 256 + half * 128:
                                     t9 * 256 + half * 128 + 128],
                                rhs, start=(t9 == 0), stop=(t9 == 8))
                    for j, (r0, nr) in enumerate(grp):
                        nc.scalar.activation(
                            cb[half][:, r0 * 128:(r0 + nr) * 128],
                            _ap(pss[j], 0, [[130, nr], [1, 128]]),
                            AF.Relu,
                            bias=bia_s[:, B_CCAM + half:B_CCAM + half + 1])
                nc.sync.dma_start(cb_dram[half, :, :], cb[half][:])

            # shunts of cb (row: mean over W%4 chunks; col: mean over H%4)
            # run on DVE/GpSimd while PE does enc + energy transposes
            for half in range(2):
                for b in range(32):
                    sl = slice(b * BL, (b + 1) * BL)
                    with nc.allow_low_precision(reason="bf16 shunt sums"):
                        src = _ap(cb[half], b * BL,
                                  [[1, 4], [128, 4], [4, 32]])
                        dst = _ap(scb_row[half], 4 * b,
                                  [[128, 4], [1, 4]])
                        nc.vector.tensor_reduce(dst, src, axis=AX.X,
                                                op=ALU.add)
                    ci = b // 8
                    part = par.tile([128, 128], F32, tag=f"cp{half}",
                                    name=f"cp{half}", bufs=2)
                    src = _ap(cb[half], b * BL, [[1, 128], [128, 4]])
                    nc.vector.tensor_reduce(part[:], src, axis=AX.X,
                                            op=ALU.add)
                    dstc = scb_col[half][:, ci * 128:(ci + 1) * 128]
                    if b % 8 == 0:
                        nc.gpsimd.tensor_copy(dstc, part[:])
                    else:
                        nc.gpsimd.tensor_tensor(dstc, dstc, part[:],
                                                ALU.add)

            # cf = relu(w_enc @ cb + b_enc)  -> [16, N]
            for bg in range(8):
                pss = [pamm.tile([16, BL], F32, tag="amm",
                                 name=f"fps{j}") for j in range(4)]
                for half in range(2):
                    for j in range(4):
                        b = bg * 4 + j
                        nc.tensor.matmul(
                            pss[j][:], wenc_s[:, half * 16:half * 16 + 16],
                            cb[half][:, b * BL:(b + 1) * BL],
                            start=(half == 0), stop=(half == 1))
                for j in range(4):
                    b = bg * 4 + j
                    nc.scalar.activation(
                        cf[:, b * BL:(b + 1) * BL], pss[j][:], AF.Relu,
                        bias=bia_s[:16, B_ENC:B_ENC + 1])

            # shunts of cf
            for b in range(32):
                with nc.allow_low_precision(reason="bf16 shunt sums"):
                    src = _ap(cf, b * BL, [[1, 4], [128, 4], [4, 32]])
                    dst = _ap(scf_row, 4 * b, [[128, 4], [1, 4]])
                    nc.vector.tensor_reduce(dst, src, axis=AX.X, op=ALU.add)
                ci = b // 8
                partf = par.tile([16, 128], F32, tag="cpf", bufs=2)
                src = _ap(cf, b * BL, [[1, 128], [128, 4]])
                nc.vector.tensor_reduce(partf[:], src, axis=AX.X, op=ALU.add)
                dstc = scf_col[:, ci * 128:(ci + 1) * 128]
                if b % 8 == 0:
                    nc.gpsimd.tensor_copy(dstc, partf[:])
                else:
                    nc.gpsimd.tensor_tensor(dstc, dstc, partf[:], ALU.add)

            # energy^T [16, 256] accumulated over 128 column-blocks.
            e_chain = [pae.tile([16, 256], F32, tag="ech", name=f"ech{c}")
                       for c in range(2)]
            for b in range(128):
                sl = slice(b * 128, (b + 1) * 128)
                tball = patr.tile([128, 272], BF, tag="tr")
                nc.tensor.matmul(tball[:, 0:128], cb[0][:, sl], idb_s[:],
                                 is_transpose=True, start=True, stop=False)
                nc.tensor.matmul(tball[:, 128:256], cb[1][:, sl], idb_s[:],
                                 is_transpose=True, start=False, stop=False)
                nc.tensor.matmul(tball[:, 256:272], cf[:, sl],
                                 idb_s[:16, :16],
                                 is_transpose=True, start=False, stop=True)
                bT = par.tile([128, 272], BF, tag="bT")
                nc.scalar.activation(bT[:], tball[:], AF.Copy)
                nc.tensor.matmul(e_chain[b % 2][:], bT[:, 256:272],
                                 bT[:, 0:256],
                                 start=(b < 2), stop=(b >= 126))

            # CCAM attention: attn = softmax(-energy) over K=16, store attn^T
            e_sb = pa.tile([16, 256], F32)
            e_tmp = pa.tile([16, 256], F32)
            nc.scalar.activation(e_tmp[:], e_chain[1][:], AF.Copy)
            nc.vector.tensor_tensor(e_sb[:], e_chain[0][:], e_tmp[:], ALU.add)
            at_half = []
            for half in range(2):
                tps = patr.tile([128, 16], F32, tag="tr")
                nc.tensor.transpose(
                    tps[:], e_sb[:, half * 128:(half + 1) * 128],
                    idf_s[:16, :16])
                e_c = par.tile([128, 16], F32, tag="ec")
                nc.vector.tensor_copy(e_c[:], tps[:])
                mn = par.tile([128, 1], F32, tag="mn")
                nc.vector.tensor_reduce(mn[:], e_c[:], axis=AX.X, op=ALU.min)
                ex = par.tile([128, 16], F32, tag="ex")
                nc.scalar.activation(ex[:], e_c[:], AF.Exp,
                                     bias=mn[:], scale=-1.0)
                sm = par.tile([128, 1], F32, tag="sm")
                nc.vector.tensor_reduce(sm[:], ex[:], axis=AX.X, op=ALU.add)
                rc = par.tile([128, 1], F32, tag="rc")
                nc.vector.reciprocal(rc[:], sm[:])
                at = par.tile([128, 16], BF, tag="at", bufs=2)
                nc.vector.tensor_scalar(at[:], ex[:], rc[:],
                                        float(scale_ccam), ALU.mult, ALU.mult)
                at_half.append(at)
                tat = patr.tile([16, 128], BF, tag="tr")
                nc.tensor.transpose(tat[:], at[:], idb_s[:])
                nc.vector.tensor_copy(
                    attnT_s[:, half * 128:(half + 1) * 128], tat[:])

            # AQ[m] = attnT @ w_m^T  -> [16, 4*128] (targets q,k,v0,v1)
            aq_ps = pae.tile([16, 512], F32, tag="ech", name="aq_ps")
            for m, (wt, mt) in enumerate([(wqb_s, 0), (wkb_s, 0),
                                          (wvb_s, 0), (wvb_s, 1)]):
                nt = 2 if wt is wvb_s else 1
                for kh in range(2):
                    nc.tensor.matmul(
                        aq_ps[:, m * 128:(m + 1) * 128], at_half[kh][:],
                        wt[:, (kh * nt + mt) * 128:(kh * nt + mt) * 128 + 128],
                        start=(kh == 0), stop=(kh == 1))
            nc.scalar.activation(aq_s[:], aq_ps[:], AF.Copy)

            # xfs = shunt(cb) + attnT @ shunt(cf)
            nc.vector.tensor_copy(scf_colb[:], scf_col[:])
            for half in range(2):
                ps = pamm.tile([128, BL], F32, tag="amm", name="xfr")
                nc.tensor.matmul(ps[:],
                                 attnT_s[:, half * 128:(half + 1) * 128],
                                 scf_row[:], start=True, stop=False)
                nc.tensor.matmul(ps[:], idb_s[:], scb_row[half][:],
                                 start=False, stop=True)
                nc.scalar.activation(xfs_row[half][:], ps[:], AF.Copy)
                ps2 = pamm.tile([128, BL], F32, tag="amm", name="xfc")
                nc.tensor.matmul(ps2[:],
                                 attnT_s[:, half * 128:(half + 1) * 128],
                                 scf_colb[:], start=True, stop=True)
                with nc.allow_low_precision(reason="bf16 xfs_col"):
                    nc.vector.tensor_tensor(xfs_col[half][:], ps2[:],
                                            scb_col[half][:], ALU.add)

          # =========================================================
          # Phase C1: axial attention (row: dir 0, col: dir 1)
          # =========================================================
          with (
              tc.tile_pool(name="pc", bufs=1) as pc,
              tc.tile_pool(name="pcr", bufs=3) as pcr,
              tc.tile_pool(name="pcm", bufs=2, space="PSUM") as pcm,
          ):
            pcmm = pcl = pcav = pcasm = pcm
            # --- interleave the two independent axial directions so one
            # direction's matmuls fill the other's pipeline latency ---
            st = {}
            for d_ in range(2):
                xfs = xfs_row if d_ == 0 else xfs_col
                qs_att = pc.tile([128, 512], BF, tag="qsa", name="qsa",
                                 bufs=2)
                ks_att = pc.tile([128, 512], BF, tag="ksa", name="ksa",
                                 bufs=2)
                vs_att = [pc.tile([128, 512], BF, tag=f"vsa{h}",
                                  name=f"vsa{h}", bufs=2)
                          for h in range(2)]
                st[d_] = dict(xfs=xfs, qs=qs_att, ks=ks_att, vs=vs_att)
                for (dst, wt, bc, nt, pidx) in [
                        ([qs_att], wqs_s, B_Q, 1, 2 * d_),
                        ([ks_att], wks_s, B_K, 1, 2 * d_ + 1),
                        (vs_att, wvs_s, B_V, 2, None)]:
                    for mt in range(nt):
                        ps = pcmm.tile([128, BL], F32, tag="cmm", bufs=2)
                        for kh in range(2):
                            nc.tensor.matmul(
                                ps[:],
                                wt[:, (kh * nt + mt) * 128:
                                   (kh * nt + mt) * 128 + 128],
                                xfs[kh][:], start=(kh == 0),
                                stop=(kh == 1 and pidx is None))
                        if pidx is not None:
                            for i in range(CH):
                                nc.tensor.matmul(
                                    ps[:, i * 128:(i + 1) * 128],
                                    post_s[:, (pidx * 4 + i) * 128:
                                           (pidx * 4 + i) * 128 + 128],
                                    interp_s[:], start=False, stop=(i == 3))
                        nc.scalar.activation(
                            dst[mt][:], ps[:], AF.Identity,
                            bias=bia_s[:, bc + mt:bc + mt + 1])

            for d_ in range(2):
                q_pack = pc.tile([128, 1024], BF, tag="qp", name="qp",
                                 bufs=2)
                k_pack = pc.tile([128, 1024], BF, tag="kp", name="kp",
                                 bufs=2)
                st[d_]["qp"] = q_pack
                st[d_]["kp"] = k_pack
                for g in range(8):
                    po, co = 32 * (g % 4), (g // 4) * 512
                    nc.sync.dma_start(
                        q_pack[po:po + 16, co:co + 512],
                        st[d_]["qs"][g * 16:(g + 1) * 16, :])
                    nc.sync.dma_start(
                        k_pack[po:po + 16, co:co + 512],
                        st[d_]["ks"][g * 16:(g + 1) * 16, :])

            for d_ in range(2):
                vt_s = pc.tile([128, 4, 256], BF, tag="vt", name="vt",
                               bufs=2)
                st[d_]["vt"] = vt_s
                for i in range(CH):
                    for hh in range(2):
                        tp = pcl.tile([128, 128], BF, tag="lps")
                        nc.tensor.transpose(
                            tp[:], st[d_]["vs"][hh][:, i * 128:(i + 1) * 128],
                            idb_s[:])
                        nc.scalar.activation(
                            vt_s[:, i, hh * 128:(hh + 1) * 128], tp[:],
                            AF.Copy)
                st[d_]["xpre"] = [
                    pc.tile([128, 512], BF, tag=f"xpre{t}",
                            name=f"xpre{t}", bufs=2) for t in range(2)]

            for i in range(CH):
                for th in range(2):
                    asm_ps = {d_: pcasm.tile([128, 128], BF, tag="asm",
                                             name=f"asm{d_}", bufs=2)
                              for d_ in range(2)}
                    for gg in range(4):
                        g = th * 4 + gg
                        po = 32 * (g % 4)
                        co = (g // 4) * 512
                        sl_gi = slice(co + i * 128, co + i * 128 + 128)
                        lps = {}
                        ets = {}
                        avs = {}
                        for d_ in range(2):
                            l_ps = pcl.tile([128, 128], F32, tag="lps",
                                            name=f"lps{d_}")
                            nc.tensor.matmul(l_ps[:],
                                             st[d_]["kp"][po:po + 16, sl_gi],
                                             st[d_]["qp"][po:po + 16, sl_gi],
                                             start=True, stop=True,
                                             tile_position=(po, 0))
                            lps[d_] = l_ps
                        for d_ in range(2):
                            e_t = pcr.tile([128, 128], BF, tag="et",
                                           name=f"et{d_}", bufs=4)
                            nc.scalar.activation(e_t[:], lps[d_][:], AF.Exp,
                                                 scale=SCALE)
                            ets[d_] = e_t
                        for d_ in range(2):
                            av_ps = pcav.tile([128, 33], F32, tag="av",
                                              name=f"av{d_}")
                            nc.tensor.matmul(
                                av_ps[:, 0:32], ets[d_][:],
                                st[d_]["vt"][:, i, g * 32:(g + 1) * 32],
                                start=True, stop=False)
                            nc.tensor.matmul(av_ps[:, 32:33], ets[d_][:],
                                             ones_s[:], start=False,
                                             stop=True)
                            avs[d_] = av_ps
                        xrns = {}
                        for d_ in range(2):
                            rcp = pcr.tile([128, 1], F32, tag="rcp",
                                           name=f"rcp{d_}", bufs=4)
                            nc.vector.reciprocal(rcp[:], avs[d_][:, 32:33])
                            xrn = pcr.tile([128, 32], BF, tag="xrn",
                                           name=f"xrn{d_}", bufs=4)
                            nc.vector.tensor_scalar(
                                xrn[:], avs[d_][:, 0:32], rcp[:], None,
                                ALU.mult)
                            xrns[d_] = xrn
                        for d_ in range(2):
                            nc.tensor.transpose(
                                asm_ps[d_][gg * 32:(gg + 1) * 32, :],
                                xrns[d_][:], idb_s[:],
                                tile_position=(0, gg * 32))
                    for d_ in range(2):
                        nc.scalar.activation(
                            st[d_]["xpre"][th][:, i * 128:(i + 1) * 128],
                            asm_ps[d_][:], AF.Relu)

            for d_ in range(2):
                wproj_d = wrow_s if d_ == 0 else wcol_s
                bcol = B_ROW if d_ == 0 else B_COL
                for mt in range(2):
                    ps = pcmm.tile([128, BL], F32, tag="cmm", bufs=2)
                    for kh in range(2):
                        nc.tensor.matmul(
                            ps[:],
                            wproj_d[:, (kh * 2 + mt) * 128:
                                    (kh * 2 + mt) * 128 + 128],
                            st[d_]["xpre"][kh][:], start=(kh == 0),
                            stop=(kh == 1))
                    nc.scalar.activation(
                        xproj[(d_, mt)][:], ps[:], AF.Identity,
                        bias=bia_s[:, bcol + mt:bcol + mt + 1])

          # =========================================================
          # Mega-loop: qkv[b] -> DW[b-1] -> PW[b-1] -> xx -> proj -> out
          # =========================================================
          with (
              tc.tile_pool(name="pf", bufs=1) as pf,
              tc.tile_pool(name="pfr", bufs=3) as pfr,
              tc.tile_pool(name="pbm", bufs=4, space="PSUM") as pbm,
              tc.tile_pool(name="pfd", bufs=3, space="PSUM") as pfd,
              tc.tile_pool(name="pfw", bufs=2, space="PSUM") as pfw,
          ):
            dwd_s = pf.tile([128, 36 * 128], BF)
            wpw_s = pf.tile([128, 4 * 256], BF)
            nc.sync.dma_start(dwd_s[:], dwd[:])
            nc.sync.dma_start(wpw_s[:], wpw[:])
            v_sb = [pf.tile([128, PSZ], BF, tag=f"v{h}", name=f"v{h}")
                    for h in range(2)]
            for t_ in v_sb:
                nc.gpsimd.memset(_ap(t_, 0, [[129 * PST, 2], [1, PST]]), 0.0)
                nc.gpsimd.memset(
                    _ap(t_, PST, [[PST, 128], [130, 2], [1, 2]]), 0.0)
            # q/k rings: 3 slots of 8 rows (2 halo + 4 data + 2 halo)
            qring = [pf.tile([128, SST], BF, name=f"qr{i}") for i in range(3)]
            kring = [pf.tile([128, SST], BF, name=f"kr{i}") for i in range(3)]
            for t_ in qring + kring:
                nc.gpsimd.memset(t_[:], 0.0)

            cbq = []
            def fetch_cb(bb):
                sl2 = slice(bb * BL, (bb + 1) * BL)
                t2 = [pfr.tile([128, BL], BF, tag=f"cbi{h}",
                               name=f"cbi{h}", bufs=4) for h in range(2)]
                nc.sync.dma_start(t2[0][:], cb_dram[0, :, sl2])
                nc.sync.dma_start(t2[1][:], cb_dram[1, :, sl2])
                cbq.append(t2)
            fetch_cb(0)
            fetch_cb(1)
            fetch_cb(2)
            for b in range(33):
              if b < 32:
                sl = slice(b * BL, (b + 1) * BL)
                if b + 3 < 32:
                    fetch_cb(b + 3)
                cbi = cbq[b]
                # q/k/v block b: 2 matmuls K=128 over cb halves + 1 K=16 (cf)
                for m, (wt, mt) in enumerate([(wqb_s, 0), (wkb_s, 0),
                                              (wvb_s, 0), (wvb_s, 1)]):
                    nt = 2 if wt is wvb_s else 1
                    ps = pbm.tile([128, BL], F32, tag="bmm",
                                  name=f"qps{m}")
                    for kh in range(2):
                        nc.tensor.matmul(
                            ps[:],
                            wt[:, (kh * nt + mt) * 128:
                               (kh * nt + mt) * 128 + 128],
                            cbi[kh][:], start=(kh == 0), stop=False)
                    nc.tensor.matmul(ps[:], aq_s[:, m * 128:(m + 1) * 128],
                                     cf[:, sl], start=False, stop=True)
                    bc = (B_Q, B_K, B_V, B_V + 1)[m]
                    if m < 2:
                        ring = qring if m == 0 else kring
                        pdst = _ap(ring[b % 3], 2 * PST + 2,
                                   [[PST, 4], [1, 128]])
                    else:
                        pdst = _ap(v_sb[m - 2], (4 * b + 1) * PST + 2,
                                   [[PST, 4], [1, 128]])
                    if m % 2 == 0:
                        nc.scalar.activation(
                            pdst, ps[:], AF.Identity,
                            bias=bia_s[:, bc:bc + 1])
                    else:
                        nc.vector.tensor_scalar(
                            pdst, ps[:], bia_s[:, bc:bc + 1],
                            None, ALU.add)
                # halo copies: slot b rows 0..1 <- slot b-1 rows 4..5;
                #              slot b-1 rows 6..7 <- slot b rows 2..3
                for ring in (qring, kring):
                    if b > 0:
                        nc.gpsimd.tensor_copy(
                            _ap(ring[b % 3], 0, [[1, 2 * PST]]),
                            _ap(ring[(b - 1) % 3], 4 * PST, [[1, 2 * PST]]))
                        nc.gpsimd.tensor_copy(
                            _ap(ring[(b - 1) % 3], 6 * PST, [[1, 2 * PST]]),
                            _ap(ring[b % 3], 2 * PST, [[1, 2 * PST]]))
                    else:
                        nc.gpsimd.memset(
                            _ap(ring[0], 0, [[1, 2 * PST]]), 0.0)
              if b == 0:
                  continue
              bp = b - 1          # tail block
              if bp == 31:
                  for ring in (qring, kring):
                      nc.gpsimd.memset(
                          _ap(ring[bp % 3], 6 * PST, [[1, 2 * PST]]), 0.0)

              # --- DW for block bp: 2-row groups j=0,1 ---
              dwg = [pfr.tile([128, 4 * 256], BF, tag=f"dwg{j}",
                              name=f"dwg{j}", bufs=2) for j in range(2)]
              for t in range(4):
                  dps = [pfd.tile([128, 260], F32, tag="dw",
                                  name=f"dps{j}", bufs=2) for j in range(2)]
                  for t9 in range(9):
                      ky, kx = divmod(t9, 3)
                      for j in range(2):
                          if t < 2:
                              src = (qring, kring)[t][bp % 3]
                              off = (1 + 2 * j + ky) * PST + kx + 1
                          else:
                              src = v_sb[t - 2]
                              off = (4 * bp + 2 * j + ky) * PST + kx + 1
                          nc.tensor.matmul(
                              _ap(dps[j], 0, [[1, 260]]),
                              dwd_s[:, (t * 9 + t9) * 128:
                                    (t * 9 + t9) * 128 + 128],
                              _ap(src, off, [[1, 260]]),
                              start=(t9 == 0), stop=(t9 == 8))
                  for j in range(2):
                      src = _ap(dps[j], 0, [[PST, 2], [1, 128]])
                      dst = _ap(dwg[j], t * 256, [[1, 256]])
                      if t % 2 == 0:
                          nc.scalar.activation(
                              dst, src, AF.Relu,
                              bias=bia_s[:, B_DW + t:B_DW + t + 1])
                      else:
                          nc.vector.tensor_scalar(
                              dst, src, bia_s[:, B_DW + t:B_DW + t + 1],
                              0.0, ALU.add, ALU.max)

              # --- PW: qo = w_pw @ dwg + b_pw ---
              pws = [pfw.tile([128, 2, 256], F32, tag="pw",
                              name=f"pws{j}") for j in range(2)]
              for mt in range(2):
                  for kt in range(4):
                      for j in range(2):
                          nc.tensor.matmul(
                              pws[j][:, mt, :],
                              wpw_s[:, kt * 256 + mt * 128:
                                    kt * 256 + mt * 128 + 128],
                              dwg[j][:, kt * 256:(kt + 1) * 256],
                              start=(kt == 0), stop=(kt == 3))
              qo_blk = [pfr.tile([128, BL], BF, tag=f"qo{mt}",
                                 name=f"qo{mt}", bufs=2) for mt in range(2)]
              for mt in range(2):
                  for j in range(2):
                      nc.scalar.activation(
                          qo_blk[mt][:, j * 256:(j + 1) * 256],
                          pws[j][:, mt, :], AF.Identity,
                          bias=bia_s[:, B_PW + mt:B_PW + mt + 1])

              # --- xx = relu(v + bcast(xrow) + bcast(xcol)) ---
              xxr = []
              for half in range(2):
                  xx = pfr.tile([128, BL], BF, tag=f"xx{half}",
                                name=f"xx{half}", bufs=2)
                  rap = _ap(xproj[(0, half)], bp * 16, [[1, 16], [0, 32]])
                  cap = _ap(xproj[(1, half)], (bp // 2) * 32,
                            [[0, 4], [0, 4], [1, 32]])
                  nc.vector.tensor_tensor(xx[:], rap, cap, ALU.add)
                  vap = _ap(v_sb[half], (4 * bp + 1) * PST + 2,
                            [[PST, 4], [1, 128]])
                  nc.vector.tensor_tensor(xx[:], xx[:], vap, ALU.add)
                  nc.vector.tensor_scalar(xx[:], xx[:], 0.0, None,
                                          ALU.max)
                  xxr.append(xx)

              # --- proj + hsig + multiply qo, DMA out ---
              for mt in range(2):
                  ps = pbm.tile([128, BL], F32, tag="bmm", name="prj")
                  for kh in range(2):
                      nc.tensor.matmul(
                          ps[:],
                          wproj_s[:, (kh * 2 + mt) * 128:
                                  (kh * 2 + mt) * 128 + 128],
                          xxr[kh][:], start=(kh == 0), stop=(kh == 1))
                  hs = pfr.tile([128, BL], BF, tag="hs", bufs=2)
                  nc.scalar.activation(
                      hs[:], ps[:], AF.Relu,
                      bias=bia_s[:, B_PROJ3 + mt:B_PROJ3 + mt + 1])
                  att_t = pfr.tile([128, BL], BF, tag="att", bufs=2)
                  nc.vector.tensor_scalar(att_t[:], hs[:], 6.0,
                                          1.0 / 6.0, ALU.min, ALU.mult)
                  ob = pfr.tile([128, BL], BF, tag="ob", bufs=2)
                  nc.vector.tensor_tensor(ob[:], att_t[:], qo_blk[mt][:],
                                          ALU.mult)
                  slp = slice(bp * BL, (bp + 1) * BL)
                  nc.gpsimd.dma_start(out[mt * 128:(mt + 1) * 128, slp],
                                      ob[:])

    nc.compile()
    return nc


def _interp_matrix():
    s, n = 16, 128
    src = np.clip((np.arange(n) + 0.5) * (s / n) - 0.5, 0.0, s - 1.0)
    i0 = np.floor(src).astype(np.int64)
    i1 = np.minimum(i0 + 1, s - 1)
    w = src - i0
    M = np.zeros((s, n), np.float64)
    np.add.at(M, (i0, np.arange(n)), 1.0 - w)
    np.add.at(M, (i1, np.arange(n)), w)
    return M


def _bf(x):
    return np.ascontiguousarray(np.asarray(x, np.float32).astype(
        ml_dtypes.bfloat16))


def prep_consts(inputs):
    """Host-side layout prep of all weight tensors (shared across cores)."""
    f = {k: np.asarray(v, np.float32) for k, v in inputs.items()}

    w3 = f["w_ccam_b"]                      # [256, 128, 3, 3]
    w3t = np.zeros((128, 9 * 256), np.float32)
    for ky in range(3):
        for kx in range(3):
            t9 = ky * 3 + kx
            w3t[:, t9 * 256:(t9 + 1) * 256] = w3[:, :, ky, kx].T
    wenc = np.zeros((128, 32), np.float32)  # w_enc [16, 256]
    for half in range(2):
        wenc[:, half * 16:(half + 1) * 16] = \
            f["w_enc"][:, half * 128:(half + 1) * 128].T

    def pack_lhsT(wm, nt):
        # wm [out, in]; returns [128, 2*nt*128]: [ci, (kh*nt+mt)*128+co]
        r = np.zeros((128, 2 * nt * 128), np.float32)
        for kh in range(2):
            for mt in range(nt):
                r[:, (kh * nt + mt) * 128:(kh * nt + mt) * 128 + 128] = \
                    wm[mt * 128:(mt + 1) * 128,
                       kh * 128:(kh + 1) * 128].T
        return r

    wq_p = pack_lhsT(f["w_q"], 1)
    wk_p = pack_lhsT(f["w_k"], 1)
    wv_p = pack_lhsT(f["w_v"], 2)
    wrow_p = pack_lhsT(f["w_row"], 2)
    wcol_p = pack_lhsT(f["w_col"], 2)
    wproj_p = pack_lhsT(f["w_proj"], 2)

    wpw_p = np.zeros((128, 4 * 256), np.float32)   # w_pw [256, 512]
    for kt in range(4):
        for mt in range(2):
            wpw_p[:, kt * 256 + mt * 128:kt * 256 + mt * 128 + 128] = \
                f["w_pw"][mt * 128:(mt + 1) * 128,
                          kt * 128:(kt + 1) * 128].T

    dwdg = np.zeros((128, 36 * 128), np.float32)   # w_dw [512,1,3,3]
    ii = np.arange(128)
    for t in range(4):
        for tap9 in range(9):
            ky, kx = divmod(tap9, 3)
            dwdg[ii, (t * 9 + tap9) * 128 + ii] = \
                f["w_dw"][t * 128 + ii, 0, ky, kx]

    post_p = np.zeros((16, 4 * 512), np.float32)
    for pidx, nm in enumerate(["pos_rowq", "pos_rowk", "pos_colq", "pos_colk"]):
        p = f[nm]                                   # [4, 128, 16]
        for i in range(4):
            post_p[:, (pidx * 4 + i) * 128:(pidx * 4 + i) * 128 + 128] = \
                p[i].T                              # [16, 128]

    biases = np.zeros((128, 20), np.float32)
    biases[:, B_CCAM + 0] = f["b_ccam_b"][:128]
    biases[:, B_CCAM + 1] = f["b_ccam_b"][128:]
    biases[:16, B_ENC] = f["b_enc"]
    biases[:, B_Q] = f["b_q"]
    biases[:, B_K] = f["b_k"]
    biases[:, B_V + 0] = f["b_v"][:128]
    biases[:, B_V + 1] = f["b_v"][128:]
    for t in range(4):
        biases[:, B_DW + t] = f["b_dw"][t * 128:(t + 1) * 128]
    biases[:, B_PW + 0] = f["b_pw"][:128]
    biases[:, B_PW + 1] = f["b_pw"][128:]
    biases[:, B_ROW + 0] = f["b_row"][:128]
    biases[:, B_ROW + 1] = f["b_row"][128:]
    biases[:, B_COL + 0] = f["b_col"][:128]
    biases[:, B_COL + 1] = f["b_col"][128:]
    biases[:, B_PROJ3 + 0] = f["b_proj"][:128] + 3.0
    biases[:, B_PROJ3 + 1] = f["b_proj"][128:] + 3.0

    return {
        "w3t": _bf(w3t), "wenc": _bf(wenc),
        "wq": _bf(wq_p), "wk": _bf(wk_p), "wv": _bf(wv_p),
        "wqs": _bf(wq_p / 32.0), "wks": _bf(wk_p / 32.0),
        "wvs": _bf(wv_p / 32.0),
        "dwd": _bf(dwdg), "wpw": _bf(wpw_p),
        "wrow": _bf(wrow_p), "wcol": _bf(wcol_p), "wproj": _bf(wproj_p),
        "post": _bf(post_p), "interpm": _bf(_interp_matrix()),
        "identb": _bf(np.eye(128)),
        "identf": np.eye(128, dtype=np.float32),
        "onesb": _bf(np.ones((128, 1))),
        "biases": np.ascontiguousarray(biases),
    }


def kernel(**inputs) -> np.ndarray:
    x = np.asarray(inputs["x"], np.float32)          # [8, 128, 128, 128]
    scale = float(np.asarray(inputs["scale_ccam"]).reshape(-1)[0])

    key = round(scale, 9)
    if key not in _CACHE:
        _CACHE[key] = build_graph(scale)
    nc = _CACHE[key]

    consts = prep_consts(inputs)
    in_maps = []
    for core in range(8):
        m = dict(consts)
        m["xb"] = np.ascontiguousarray(x[core].reshape(128, N))
        in_maps.append(m)

    res = run_bass_kernel_spmd(nc, in_maps, core_ids=list(range(8)))
    outs = [res.results[i]["out"].reshape(256, 128, 128) for i in range(8)]
    return np.stack(outs).astype(np.float32)


if __name__ == "__main__":
    rng = np.random.default_rng(0)
    demo = {"x": rng.standard_norm_((8, 128, 128, 128))} if False else None
    print("kernel module OK")


# revision 41
# speedup vs baseline: 1.0066x; 1.0025x over previous
"""Trainium2 Bass kernel for nn_Align_54279796687162 (sparse_attention).

Pure data parallel: one sample per NeuronCore (B=8 over 8 cores).
v3: all-bf16 datapath. cb/cf SBUF-resident; shunts computed in phase A via
linearity (shunt(xf) = shunt(cb) + attnT @ shunt(cf)); q/k/v computed
directly from cb/cf (q = wq@cb + (attnT@wq^T)^T@cf); single PE-bound
mega-loop: qkv -> DW (diag matmuls) -> PW -> xx -> proj -> out.
"""

import numpy as np
import ml_dtypes

import concourse.bass as bass
import concourse.mybir as mybir
import concourse.tile as tile
from concourse import bacc
from concourse.bass_utils import run_bass_kernel_spmd

BF = mybir.dt.bfloat16
F32 = mybir.dt.float32
AF = mybir.ActivationFunctionType
ALU = mybir.AluOpType
AX = mybir.AxisListType

H = W = 128
N = H * W            # 16384
BL = 512             # block size (4 rows * 128)
CH = 4               # chunks
SCALE = 0.25         # KD ** -0.5
PST = 132            # padded row stride for q/k/v (DW conv layout)
PSZ = PST * 130      # padded tensor size per partition
SST = 8 * PST        # q/k ring slot stride (8 rows: 4 data + 2+2 halo)

# bias column map in the packed [128, 20] f32 bias tile
B_CCAM, B_ENC, B_Q, B_K, B_V, B_DW, B_PW, B_ROW, B_COL, B_PROJ3 = (
    0, 2, 3, 4, 5, 7, 11, 13, 15, 17)

_CACHE = {}


def _ap(base, extra_off, free_dims):
    """Build an AP from a tile's base AP with custom free dims."""
    b = base[:]
    return bass.AP(b.tensor, b.offset + extra_off, [list(b.ap[0])] + free_dims)


def build_graph(scale_ccam: float):
    nc = bacc.Bacc(None, target_bir_lowering=False)

    xb = nc.dram_tensor("xb", [128, N], F32, kind="ExternalInput")
    w3t = nc.dram_tensor("w3t", [128, 9 * 256], BF, kind="ExternalInput")
    wenc = nc.dram_tensor("wenc", [128, 32], BF, kind="ExternalInput")
    wq = nc.dram_tensor("wq", [128, 256], BF, kind="ExternalInput")
    wk = nc.dram_tensor("wk", [128, 256], BF, kind="ExternalInput")
    wv = nc.dram_tensor("wv", [128, 512], BF, kind="ExternalInput")
    dwd = nc.dram_tensor("dwd", [128, 36 * 128], BF, kind="ExternalInput")
    wpw = nc.dram_tensor("wpw", [128, 4 * 256], BF, kind="ExternalInput")
    wqs = nc.dram_tensor("wqs", [128, 256], BF, kind="ExternalInput")
    wks = nc.dram_tensor("wks", [128, 256], BF, kind="ExternalInput")
    wvs = nc.dram_tensor("wvs", [128, 512], BF, kind="ExternalInput")
    wrow = nc.dram_tensor("wrow", [128, 512], BF, kind="ExternalInput")
    wcol = nc.dram_tensor("wcol", [128, 512], BF, kind="ExternalInput")
    wproj = nc.dram_tensor("wproj", [128, 512], BF, kind="ExternalInput")
    post = nc.dram_tensor("post", [16, 4 * 512], BF, kind="ExternalInput")
    interpm = nc.dram_tensor("interpm", [16, 128], BF, kind="ExternalInput")
    identb = nc.dram_tensor("identb", [128, 128], BF, kind="ExternalInput")
    identf = nc.dram_tensor("identf", [128, 128], F32, kind="ExternalInput")
    onesb = nc.dram_tensor("onesb", [128, 1], BF, kind="ExternalInput")
    biases = nc.dram_tensor("biases", [128, 20], F32, kind="ExternalInput")

    cb_dram = nc.dram_tensor("cb_dram", [2, 128, N], BF, kind="Internal")
    out = nc.dram_tensor("out", [256, N], F32, kind="ExternalOutput")

    with tile.TileContext(nc) as tc:
      with tc.tile_pool(name="cst", bufs=1) as cst:
        w3_s = cst.tile([128, 9 * 256], BF)
        idf_s = cst.tile([128, 128], F32)
        nc.sync.dma_start(w3_s[:], w3t[:])
        nc.sync.dma_start(idf_s[:], identf[:])
        wenc_s = cst.tile([128, 32], BF)
        wqb_s = cst.tile([128, 256], BF)
        wkb_s = cst.tile([128, 256], BF)
        wvb_s = cst.tile([128, 512], BF)
        wproj_s = cst.tile([128, 512], BF)
        idb_s = cst.tile([128, 128], BF)
        ones_s = cst.tile([128, 1], BF)
        bia_s = cst.tile([128, 20], F32)
        for t, d in [(wenc_s, wenc), (wqb_s, wq), (wkb_s, wk), (wvb_s, wv),
                     (wproj_s, wproj), (idb_s, identb),
                     (ones_s, onesb), (bia_s, biases)]:
            nc.sync.dma_start(t[:], d[:])

        wqs_s = cst.tile([128, 256], BF)
        wks_s = cst.tile([128, 256], BF)
        wvs_s = cst.tile([128, 512], BF)
        wrow_s = cst.tile([128, 512], BF)
        wcol_s = cst.tile([128, 512], BF)
        post_s = cst.tile([16, 4 * 512], BF)
        interp_s = cst.tile([16, 128], BF)
        for t, d in [(wqs_s, wqs), (wks_s, wks), (wvs_s, wvs),
                     (wrow_s, wrow), (wcol_s, wcol), (post_s, post),
                     (interp_s, interpm)]:
            nc.sync.dma_start(t[:], d[:])
        attnT_s = cst.tile([16, 256], BF)
        aq_s = cst.tile([16, 512], BF)       # (attnT @ w{q,k,v}^T) per target
        xfs_row = [cst.tile([128, 512], BF, tag=f"xfsr{h}", name=f"xfsr{h}") for h in range(2)]
        xfs_col = [cst.tile([128, 512], BF, tag=f"xfsc{h}", name=f"xfsc{h}") for h in range(2)]
        xproj = {(d_, t_): cst.tile([128, 512], BF, tag=f"xp{d_}{t_}", name=f"xp{d_}{t_}")
                 for d_ in range(2) for t_ in range(2)}

        with tc.tile_pool(name="pmid", bufs=1) as pmid:
          cf = pmid.tile([16, N], BF)

          # =========================================================
          # Phase A: conv3x3 -> cb ; cf ; shunts of cb/cf ; energy ;
          #          ccam softmax ; xfs assembly ; AQ
          # =========================================================
          with (
              tc.tile_pool(name="pa", bufs=1) as pa,
              tc.tile_pool(name="par", bufs=3) as par,
              tc.tile_pool(name="pamm", bufs=4, space="PSUM") as pamm,
              tc.tile_pool(name="patr", bufs=2, space="PSUM") as patr,
              tc.tile_pool(name="pae", bufs=2, space="PSUM") as pae,
          ):
            cb = [pa.tile([128, N], BF, tag=f"cb{h}", name=f"cb{h}")
                  for h in range(2)]
            xpad = pa.tile([128, 130 * 130], BF)

            # zero only the pad border of xpad; DMA x (f32->bf16) straight
            # into the interior, 32 rows at a time
            nc.vector.memset(_ap(xpad, 0, [[1, 130]]), 0.0)
            nc.vector.memset(_ap(xpad, 129 * 130, [[1, 130]]), 0.0)
            nc.vector.memset(_ap(xpad, 130, [[130, 128], [129, 2]]), 0.0)
            for rc in range(4):
                nc.gpsimd.dma_start(
                    _ap(xpad, 131 + rc * 32 * 130, [[130, 32], [1, 128]]),
                    _ap(xb, rc * 32 * 128, [[128, 32], [1, 128]]))

            scb_row = [pa.tile([128, 512], BF, tag=f"sbr{h}", name=f"sbr{h}")
                       for h in range(2)]
            scb_col = [pa.tile([128, 512], F32, tag=f"sbc{h}", name=f"sbc{h}")
                       for h in range(2)]
            scf_row = pa.tile([16, 512], BF)
            scf_col = pa.tile([16, 512], F32)
            scf_colb = pa.tile([16, 512], BF)

            # conv3x3: contiguous padded windows (junk cols stripped by
            # the ACT extraction copy), tap-major over 4-block psum groups
            cblk = [(r0, 3) for r0 in range(0, 126, 3)] + [(126, 2)]
            for half in range(2):
                for g0 in range(0, len(cblk), 2):
                    grp = cblk[g0:g0 + 2]
                    pss = [pamm.tile([128, BL], F32, tag="amm",
                                     name=f"cps{j}")
                           for j in range(len(grp))]
                    for t9 in range(9):
                        ky, kx = divmod(t9, 3)
                        for j, (r0, nr) in enumerate(grp):
                            rhs = _ap(xpad, (r0 + ky) * 130 + kx,
                                      [[1, nr * 130 - 2]])
                            nc.tensor.matmul(
                                _ap(pss[j], 0, [[1, nr * 130 - 2]]),
                                w3_s[:, t9 * 256 + half * 128:
                                     t9 * 256 + half * 128 + 128],
                                rhs, start=(t9 == 0), stop=(t9 == 8))
                    for j, (r0, nr) in enumerate(grp):
                        nc.scalar.activation(
                            cb[half][:, r0 * 128:(r0 + nr) * 128],
                            _ap(pss[j], 0, [[130, nr], [1, 128]]),
                            AF.Relu,
                            bias=bia_s[:, B_CCAM + half:B_CCAM + half + 1])
                nc.sync.dma_start(cb_dram[half, :, :], cb[half][:])

            # shunts of cb (row: mean over W%4 chunks; col: mean over H%4)
            # run on DVE/GpSimd while PE does enc + energy transposes
            for half in range(2):
                for b in range(32):
                    sl = slice(b * BL, (b + 1) * BL)
                    with nc.allow_low_precision(reason="bf16 shunt sums"):
                        src = _ap(cb[half], b * BL,
                                  [[1, 4], [128, 4], [4, 32]])
                        dst = _ap(scb_row[half], 4 * b,
                                  [[128, 4], [1, 4]])
                        nc.vector.tensor_reduce(dst, src, axis=AX.X,
                                                op=ALU.add)
                    ci = b // 8
                    part = par.tile([128, 128], F32, tag=f"cp{half}",
                                    name=f"cp{half}", bufs=2)
                    src = _ap(cb[half], b * BL, [[1, 128], [128, 4]])
                    nc.vector.tensor_reduce(part[:], src, axis=AX.X,
                                            op=ALU.add)
                    dstc = scb_col[half][:, ci * 128:(ci + 1) * 128]
                    if b % 8 == 0:
                        nc.gpsimd.tensor_copy(dstc, part[:])
                    else:
                        nc.gpsimd.tensor_tensor(dstc, dstc, part[:],
                                                ALU.add)

            # cf = relu(w_enc @ cb + b_enc)  -> [16, N]
            for bg in range(8):
                pss = [pamm.tile([16, BL], F32, tag="amm",
                                 name=f"fps{j}") for j in range(4)]
                for half in range(2):
                    for j in range(4):
                        b = bg * 4 + j
                        nc.tensor.matmul(
                            pss[j][:], wenc_s[:, half * 16:half * 16 + 16],
                            cb[half][:, b * BL:(b + 1) * BL],
                            start=(half == 0), stop=(half == 1))
                for j in range(4):
                    b = bg * 4 + j
                    nc.scalar.activation(
                        cf[:, b * BL:(b + 1) * BL], pss[j][:], AF.Relu,
                        bias=bia_s[:16, B_ENC:B_ENC + 1])

            # shunts of cf
            for b in range(32):
                with nc.allow_low_precision(reason="bf16 shunt sums"):
                    src = _ap(cf, b * BL, [[1, 4], [128, 4], [4, 32]])
                    dst = _ap(scf_row, 4 * b, [[128, 4], [1, 4]])
                    nc.vector.tensor_reduce(dst, src, axis=AX.X, op=ALU.add)
                ci = b // 8
                partf = par.tile([16, 128], F32, tag="cpf", bufs=2)
                src = _ap(cf, b * BL, [[1, 128], [128, 4]])
                nc.vector.tensor_reduce(partf[:], src, axis=AX.X, op=ALU.add)
                dstc = scf_col[:, ci * 128:(ci + 1) * 128]
                if b % 8 == 0:
                    nc.gpsimd.tensor_copy(dstc, partf[:])
                else:
                    nc.gpsimd.tensor_tensor(dstc, dstc, partf[:], ALU.add)

            # energy^T [16, 256] accumulated over 128 column-blocks.
            e_chain = [pae.tile([16, 256], F32, tag="ech", name=f"ech{c}")
                       for c in range(2)]
            for b in range(128):
                sl = slice(b * 128, (b + 1) * 128)
                tball = patr.tile([128, 272], BF, tag="tr")
                nc.tensor.matmul(tball[:, 0:128], cb[0][:, sl], idb_s[:],
                                 is_transpose=True, start=True, stop=False)
                nc.tensor.matmul(tball[:, 128:256], cb[1][:, sl], idb_s[:],
                                 is_transpose=True, start=False, stop=False)
                nc.tensor.matmul(tball[:, 256:272], cf[:, sl],
                                 idb_s[:16, :16],
                                 is_transpose=True, start=False, stop=True)
                bT = par.tile([128, 272], BF, tag="bT")
                nc.scalar.activation(bT[:], tball[:], AF.Copy)
                nc.tensor.matmul(e_chain[b % 2][:], bT[:, 256:272],
                                 bT[:, 0:256],
                                 start=(b < 2), stop=(b >= 126))

            # CCAM attention: attn = softmax(-energy) over K=16, store attn^T
            e_sb = pa.tile([16, 256], F32)
            e_tmp = pa.tile([16, 256], F32)
            nc.scalar.activation(e_tmp[:], e_chain[1][:], AF.Copy)
            nc.vector.tensor_tensor(e_sb[:], e_chain[0][:], e_tmp[:], ALU.add)
            at_half = []
            for half in range(2):
                tps = patr.tile([128, 16], F32, tag="tr")
                nc.tensor.transpose(
                    tps[:], e_sb[:, half * 128:(half + 1) * 128],
                    idf_s[:16, :16])
                e_c = par.tile([128, 16], F32, tag="ec")
                nc.vector.tensor_copy(e_c[:], tps[:])
                mn = par.tile([128, 1], F32, tag="mn")
                nc.vector.tensor_reduce(mn[:], e_c[:], axis=AX.X, op=ALU.min)
                ex = par.tile([128, 16], F32, tag="ex")
                nc.scalar.activation(ex[:], e_c[:], AF.Exp,
                                     bias=mn[:], scale=-1.0)
                sm = par.tile([128, 1], F32, tag="sm")
                nc.vector.tensor_reduce(sm[:], ex[:], axis=AX.X, op=ALU.add)
                rc = par.tile([128, 1], F32, tag="rc")
                nc.vector.reciprocal(rc[:], sm[:])
                at = par.tile([128, 16], BF, tag="at", bufs=2)
                nc.vector.tensor_scalar(at[:], ex[:], rc[:],
                                        float(scale_ccam), ALU.mult, ALU.mult)
                at_half.append(at)
                tat = patr.tile([16, 128], BF, tag="tr")
                nc.tensor.transpose(tat[:], at[:], idb_s[:])
                nc.vector.tensor_copy(
                    attnT_s[:, half * 128:(half + 1) * 128], tat[:])

            # AQ[m] = attnT @ w_m^T  -> [16, 4*128] (targets q,k,v0,v1)
            aq_ps = pae.tile([16, 512], F32, tag="ech", name="aq_ps")
            for m, (wt, mt) in enumerate([(wqb_s, 0), (wkb_s, 0),
                                          (wvb_s, 0), (wvb_s, 1)]):
                nt = 2 if wt is wvb_s else 1
                for kh in range(2):
                    nc.tensor.matmul(
                        aq_ps[:, m * 128:(m + 1) * 128], at_half[kh][:],
                        wt[:, (kh * nt + mt) * 128:(kh * nt + mt) * 128 + 128],
                        start=(kh == 0), stop=(kh == 1))
            nc.scalar.activation(aq_s[:], aq_ps[:], AF.Copy)

            # xfs = shunt(cb) + attnT @ shunt(cf)
            nc.vector.tensor_copy(scf_colb[:], scf_col[:])
            for half in range(2):
                ps = pamm.tile([128, BL], F32, tag="amm", name="xfr")
                nc.tensor.matmul(ps[:],
                                 attnT_s[:, half * 128:(half + 1) * 128],
                                 scf_row[:], start=True, stop=False)
                nc.tensor.matmul(ps[:], idb_s[:], scb_row[half][:],
                                 start=False, stop=True)
                nc.scalar.activation(xfs_row[half][:], ps[:], AF.Copy)
                ps2 = pamm.tile([128, BL], F32, tag="amm", name="xfc")
                nc.tensor.matmul(ps2[:],
                                 attnT_s[:, half * 128:(half + 1) * 128],
                                 scf_colb[:], start=True, stop=True)
                with nc.allow_low_precision(reason="bf16 xfs_col"):
                    nc.vector.tensor_tensor(xfs_col[half][:], ps2[:],
                                            scb_col[half][:], ALU.add)

          # =========================================================
          # Phase C1: axial attention (row: dir 0, col: dir 1)
          # =========================================================
          with (
              tc.tile_pool(name="pc", bufs=1) as pc,
              tc.tile_pool(name="pcr", bufs=3) as pcr,
              tc.tile_pool(name="pcm", bufs=2, space="PSUM") as pcm,
          ):
            pcmm = pcl = pcav = pcasm = pcm
            # --- interleave the two independent axial directions so one
            # direction's matmuls fill the other's pipeline latency ---
            st = {}
            for d_ in range(2):
                xfs = xfs_row if d_ == 0 else xfs_col
                qs_att = pc.tile([128, 512], BF, tag="qsa", name="qsa",
                                 bufs=2)
                ks_att = pc.tile([128, 512], BF, tag="ksa", name="ksa",
                                 bufs=2)
                vs_att = [pc.tile([128, 512], BF, tag=f"vsa{h}",
                                  name=f"vsa{h}", bufs=2)
                          for h in range(2)]
                st[d_] = dict(xfs=xfs, qs=qs_att, ks=ks_att, vs=vs_att)
                for (dst, wt, bc, nt, pidx) in [
                        ([qs_att], wqs_s, B_Q, 1, 2 * d_),
                        ([ks_att], wks_s, B_K, 1, 2 * d_ + 1),
                        (vs_att, wvs_s, B_V, 2, None)]:
                    for mt in range(nt):
                        ps = pcmm.tile([128, BL], F32, tag="cmm", bufs=2)
                        for kh in range(2):
                            nc.tensor.matmul(
                                ps[:],
                                wt[:, (kh * nt + mt) * 128:
                                   (kh * nt + mt) * 128 + 128],
                                xfs[kh][:], start=(kh == 0),
                                stop=(kh == 1 and pidx is None))
                        if pidx is not None:
                            for i in range(CH):
                                nc.tensor.matmul(
                                    ps[:, i * 128:(i + 1) * 128],
                                    post_s[:, (pidx * 4 + i) * 128:
                                           (pidx * 4 + i) * 128 + 128],
                                    interp_s[:], start=False, stop=(i == 3))
                        nc.scalar.activation(
                            dst[mt][:], ps[:], AF.Identity,
                            bias=bia_s[:, bc + mt:bc + mt + 1])

            for d_ in range(2):
                q_pack = pc.tile([128, 1024], BF, tag="qp", name="qp",
                                 bufs=2)
                k_pack = pc.tile([128, 1024], BF, tag="kp", name="kp",
                                 bufs=2)
                st[d_]["qp"] = q_pack
                st[d_]["kp"] = k_pack
                for g in range(8):
                    po, co = 32 * (g % 4), (g // 4) * 512
                    nc.sync.dma_start(
                        q_pack[po:po + 16, co:co + 512],
                        st[d_]["qs"][g * 16:(g + 1) * 16, :])
                    nc.sync.dma_start(
                        k_pack[po:po + 16, co:co + 512],
                        st[d_]["ks"][g * 16:(g + 1) * 16, :])

            for d_ in range(2):
                vt_s = pc.tile([128, 4, 256], BF, tag="vt", name="vt",
                               bufs=2)
                st[d_]["vt"] = vt_s
                for i in range(CH):
                    for hh in range(2):
                        tp = pcl.tile([128, 128], BF, tag="lps")
                        nc.tensor.transpose(
                            tp[:], st[d_]["vs"][hh][:, i * 128:(i + 1) * 128],
                            idb_s[:])
                        nc.scalar.activation(
                            vt_s[:, i, hh * 128:(hh + 1) * 128], tp[:],
                            AF.Copy)
                st[d_]["xpre"] = [
                    pc.tile([128, 512], BF, tag=f"xpre{t}",
                            name=f"xpre{t}", bufs=2) for t in range(2)]

            for i in range(CH):
                for th in range(2):
                    asm_ps = {d_: pcasm.tile([128, 128], BF, tag="asm",
                                             name=f"asm{d_}", bufs=2)
                              for d_ in range(2)}
                    for gg in range(4):
                        g = th * 4 + gg
                        po = 32 * (g % 4)
                        co = (g // 4) * 512
                        sl_gi = slice(co + i * 128, co + i * 128 + 128)
                        lps = {}
                        ets = {}
                        avs = {}
                        for d_ in range(2):
                            l_ps = pcl.tile([128, 128], F32, tag="lps",
                                            name=f"lps{d_}")
                            nc.tensor.matmul(l_ps[:],
                                             st[d_]["kp"][po:po + 16, sl_gi],
                                             st[d_]["qp"][po:po + 16, sl_gi],
                                             start=True, stop=True,
                                             tile_position=(po, 0))
                            lps[d_] = l_ps
                        for d_ in range(2):
                            e_t = pcr.tile([128, 128], BF, tag="et",
                                           name=f"et{d_}", bufs=4)
                            nc.scalar.activation(e_t[:], lps[d_][:], AF.Exp,
                                                 scale=SCALE)
                            ets[d_] = e_t
                        for d_ in range(2):
                            av_ps = pcav.tile([128, 33], F32, tag="av",
                                              name=f"av{d_}")
                            nc.tensor.matmul(
                                av_ps[:, 0:32], ets[d_][:],
                                st[d_]["vt"][:, i, g * 32:(g + 1) * 32],
                                start=True, stop=False)
                            nc.tensor.matmul(av_ps[:, 32:33], ets[d_][:],
                                             ones_s[:], start=False,
                                             stop=True)
                            avs[d_] = av_ps
                        xrns = {}
                        for d_ in range(2):
                            rcp = pcr.tile([128, 1], F32, tag="rcp",
                                           name=f"rcp{d_}", bufs=4)
                            nc.vector.reciprocal(rcp[:], avs[d_][:, 32:33])
                            xrn = pcr.tile([128, 32], BF, tag="xrn",
                                           name=f"xrn{d_}", bufs=4)
                            nc.vector.tensor_scalar(
                                xrn[:], avs[d_][:, 0:32], rcp[:], None,
                                ALU.mult)
                            xrns[d_] = xrn
                        for d_ in range(2):
                            nc.tensor.transpose(
                                asm_ps[d_][gg * 32:(gg + 1) * 32, :],
                                xrns[d_][:], idb_s[:],
                                tile_position=(0, gg * 32))
                    for d_ in range(2):
                        nc.scalar.activation(
                            st[d_]["xpre"][th][:, i * 128:(i + 1) * 128],
                            asm_ps[d_][:], AF.Relu)

            for d_ in range(2):
                wproj_d = wrow_s if d_ == 0 else wcol_s
                bcol = B_ROW if d_ == 0 else B_COL
                for mt in range(2):
                    ps = pcmm.tile([128, BL], F32, tag="cmm", bufs=2)
                    for kh in range(2):
                        nc.tensor.matmul(
                            ps[:],
                            wproj_d[:, (kh * 2 + mt) * 128:
                                    (kh * 2 + mt) * 128 + 128],
                            st[d_]["xpre"][kh][:], start=(kh == 0),
                            stop=(kh == 1))
                    nc.scalar.activation(
                        xproj[(d_, mt)][:], ps[:], AF.Identity,
                        bias=bia_s[:, bcol + mt:bcol + mt + 1])

          # =========================================================
          # Mega-loop: qkv[b] -> DW[b-1] -> PW[b-1] -> xx -> proj -> out
          # =========================================================
          with (
              tc.tile_pool(name="pf", bufs=1) as pf,
              tc.tile_pool(name="pfr", bufs=3) as pfr,
              tc.tile_pool(name="pbm", bufs=4, space="PSUM") as pbm,
              tc.tile_pool(name="pfd", bufs=3, space="PSUM") as pfd,
              tc.tile_pool(name="pfw", bufs=2, space="PSUM") as pfw,
          ):
            dwd_s = pf.tile([128, 36 * 128], BF)
            wpw_s = pf.tile([128, 4 * 256], BF)
            nc.sync.dma_start(dwd_s[:], dwd[:])
            nc.sync.dma_start(wpw_s[:], wpw[:])
            v_sb = [pf.tile([128, PSZ], BF, tag=f"v{h}", name=f"v{h}")
                    for h in range(2)]
            for t_ in v_sb:
                nc.gpsimd.memset(_ap(t_, 0, [[129 * PST, 2], [1, PST]]), 0.0)
                nc.gpsimd.memset(
                    _ap(t_, PST, [[PST, 128], [130, 2], [1, 2]]), 0.0)
            # q/k rings: 3 slots of 8 rows (2 halo + 4 data + 2 halo)
            qring = [pf.tile([128, SST], BF, name=f"qr{i}") for i in range(3)]
            kring = [pf.tile([128, SST], BF, name=f"kr{i}") for i in range(3)]
            for t_ in qring + kring:
                nc.gpsimd.memset(t_[:], 0.0)

            cbq = []
            def fetch_cb(bb):
                sl2 = slice(bb * BL, (bb + 1) * BL)
                t2 = [pfr.tile([128, BL], BF, tag=f"cbi{h}",
                               name=f"cbi{h}", bufs=4) for h in range(2)]
                nc.sync.dma_start(t2[0][:], cb_dram[0, :, sl2])
                nc.sync.dma_start(t2[1][:], cb_dram[1, :, sl2])
                cbq.append(t2)
            fetch_cb(0)
            fetch_cb(1)
            fetch_cb(2)
            for b in range(33):
              if b < 32:
                sl = slice(b * BL, (b + 1) * BL)
                if b + 3 < 32:
                    fetch_cb(b + 3)
                cbi = cbq[b]
                # q/k/v block b: 2 matmuls K=128 over cb halves + 1 K=16 (cf)
                for m, (wt, mt) in enumerate([(wqb_s, 0), (wkb_s, 0),
                                              (wvb_s, 0), (wvb_s, 1)]):
                    nt = 2 if wt is wvb_s else 1
                    ps = pbm.tile([128, BL], F32, tag="bmm",
                                  name=f"qps{m}")
                    for kh in range(2):
                        nc.tensor.matmul(
                            ps[:],
                            wt[:, (kh * nt + mt) * 128:
                               (kh * nt + mt) * 128 + 128],
                            cbi[kh][:], start=(kh == 0), stop=False)
                    nc.tensor.matmul(ps[:], aq_s[:, m * 128:(m + 1) * 128],
                                     cf[:, sl], start=False, stop=True)
                    bc = (B_Q, B_K, B_V, B_V + 1)[m]
                    if m < 2:
                        ring = qring if m == 0 else kring
                        pdst = _ap(ring[b % 3], 2 * PST + 2,
                                   [[PST, 4], [1, 128]])
                    else:
                        pdst = _ap(v_sb[m - 2], (4 * b + 1) * PST + 2,
                                   [[PST, 4], [1, 128]])
                    if m % 2 == 0:
                        nc.scalar.activation(
                            pdst, ps[:], AF.Identity,
                            bias=bia_s[:, bc:bc + 1])
                    else:
                        nc.vector.tensor_scalar(
                            pdst, ps[:], bia_s[:, bc:bc + 1],
                            None, ALU.add)
                # halo copies: slot b rows 0..1 <- slot b-1 rows 4..5;
                #              slot b-1 rows 6..7 <- slot b rows 2..3
                for ring in (qring, kring):
                    if b > 0:
                        nc.gpsimd.tensor_copy(
                            _ap(ring[b % 3], 0, [[1, 2 * PST]]),
                            _ap(ring[(b - 1) % 3], 4 * PST, [[1, 2 * PST]]))
                        nc.gpsimd.tensor_copy(
                            _ap(ring[(b - 1) % 3], 6 * PST, [[1, 2 * PST]]),
                            _ap(ring[b % 3], 2 * PST, [[1, 2 * PST]]))
                    else:
                        nc.gpsimd.memset(
                            _ap(ring[0], 0, [[1, 2 * PST]]), 0.0)
              if b == 0:
                  continue
              bp = b - 1          # tail block
              if bp == 31:
                  for ring in (qring, kring):
                      nc.gpsimd.memset(
                          _ap(ring[bp % 3], 6 * PST, [[1, 2 * PST]]), 0.0)

              # --- DW for block bp: 2-row groups j=0,1 ---
              dwg = [pfr.tile([128, 4 * 256], BF, tag=f"dwg{j}",
                              name=f"dwg{j}", bufs=2) for j in range(2)]
              for t in range(4):
                  dps = [pfd.tile([128, 260], F32, tag="dw",
                                  name=f"dps{j}", bufs=2) for j in range(2)]
                  for t9 in range(9):
                      ky, kx = divmod(t9, 3)
                      for j in range(2):
                          if t < 2:
                              src = (qring, kring)[t][bp % 3]
                              off = (1 + 2 * j + ky) * PST + kx + 1
                          else:
                              src = v_sb[t - 2]
                              off = (4 * bp + 2 * j + ky) * PST + kx + 1
                          nc.tensor.matmul(
                              _ap(dps[j], 0, [[1, 260]]),
                              dwd_s[:, (t * 9 + t9) * 128:
                                    (t * 9 + t9) * 128 + 128],
                              _ap(src, off, [[1, 260]]),
                              start=(t9 == 0), stop=(t9 == 8))
                  for j in range(2):
                      src = _ap(dps[j], 0, [[PST, 2], [1, 128]])
                      dst = _ap(dwg[j], t * 256, [[1, 256]])
                      if t % 2 == 0:
                          nc.scalar.activation(
                              dst, src, AF.Relu,
                              bias=bia_s[:, B_DW + t:B_DW + t + 1])
                      else:
                          nc.vector.tensor_scalar(
                              dst, src, bia_s[:, B_DW + t:B_DW + t + 1],
                              0.0, ALU.add, ALU.max)

              # --- PW: qo = w_pw @ dwg + b_pw ---
              pws = [pfw.tile([128, 2, 256], F32, tag="pw",
                              name=f"pws{j}") for j in range(2)]
              for mt in range(2):
                  for kt in range(4):
                      for j in range(2):
                          nc.tensor.matmul(
                              pws[j][:, mt, :],
                              wpw_s[:, kt * 256 + mt * 128:
                                    kt * 256 + mt * 128 + 128],
                              dwg[j][:, kt * 256:(kt + 1) * 256],
                              start=(kt == 0), stop=(kt == 3))
              qo_blk = [pfr.tile([128, BL], BF, tag=f"qo{mt}",
                                 name=f"qo{mt}", bufs=2) for mt in range(2)]
              for mt in range(2):
                  for j in range(2):
                      nc.scalar.activation(
                          qo_blk[mt][:, j * 256:(j + 1) * 256],
                          pws[j][:, mt, :], AF.Identity,
                          bias=bia_s[:, B_PW + mt:B_PW + mt + 1])

              # --- xx = relu(v + bcast(xrow) + bcast(xcol)) ---
              xxr = []
              for half in range(2):
                  xx = pfr.tile([128, BL], BF, tag=f"xx{half}",
                                name=f"xx{half}", bufs=2)
                  rap = _ap(xproj[(0, half)], bp * 16, [[1, 16], [0, 32]])
                  cap = _ap(xproj[(1, half)], (bp // 2) * 32,
                            [[0, 4], [0, 4], [1, 32]])
                  nc.vector.tensor_tensor(xx[:], rap, cap, ALU.add)
                  vap = _ap(v_sb[half], (4 * bp + 1) * PST + 2,
                            [[PST, 4], [1, 128]])
                  nc.vector.tensor_tensor(xx[:], xx[:], vap, ALU.add)
                  nc.vector.tensor_scalar(xx[:], xx[:], 0.0, None,
                                          ALU.max)
                  xxr.append(xx)

              # --- proj + hsig + multiply qo, DMA out ---
              for mt in range(2):
                  ps = pbm.tile([128, BL], F32, tag="bmm", name="prj")
                  for kh in range(2):
                      nc.tensor.matmul(
                          ps[:],
                          wproj_s[:, (kh * 2 + mt) * 128:
                                  (kh * 2 + mt) * 128 + 128],
                          xxr[kh][:], start=(kh == 0), stop=(kh == 1))
                  hs = pfr.tile([128, BL], BF, tag="hs", bufs=2)
                  nc.scalar.activation(
                      hs[:], ps[:], AF.Relu,
                      bias=bia_s[:, B_PROJ3 + mt:B_PROJ3 + mt + 1])
                  att_t = pfr.tile([128, BL], BF, tag="att", bufs=2)
                  nc.vector.tensor_scalar(att_t[:], hs[:], 6.0,
                                          1.0 / 6.0, ALU.min, ALU.mult)
                  ob = pfr.tile([128, BL], BF, tag="ob", bufs=2)
                  nc.vector.tensor_tensor(ob[:], att_t[:], qo_blk[mt][:],
                                          ALU.mult)
                  slp = slice(bp * BL, (bp + 1) * BL)
                  nc.gpsimd.dma_start(out[mt * 128:(mt + 1) * 128, slp],
                                      ob[:])

    nc.compile()
    return nc


def _interp_matrix():
    s, n = 16, 128
    src = np.clip((np.arange(n) + 0.5) * (s / n) - 0.5, 0.0, s - 1.0)
    i0 = np.floor(src).astype(np.int64)
    i1 = np.minimum(i0 + 1, s - 1)
    w = src - i0
    M = np.zeros((s, n), np.float64)
    np.add.at(M, (i0, np.arange(n)), 1.0 - w)
    np.add.at(M, (i1, np.arange(n)), w)
    return M


def _bf(x):
    return np.ascontiguousarray(np.asarray(x, np.float32).astype(
        ml_dtypes.bfloat16))


def prep_consts(inputs):
    """Host-side layout prep of all weight tensors (shared across cores)."""
    f = {k: np.asarray(v, np.float32) for k, v in inputs.items()}

    w3 = f["w_ccam_b"]                      # [256, 128, 3, 3]
    w3t = np.zeros((128, 9 * 256), np.float32)
    for ky in range(3):
        for kx in range(3):
            t9 = ky * 3 + kx
            w3t[:, t9 * 256:(t9 + 1) * 256] = w3[:, :, ky, kx].T
    wenc = np.zeros((128, 32), np.float32)  # w_enc [16, 256]
    for half in range(2):
        wenc[:, half * 16:(half + 1) * 16] = \
            f["w_enc"][:, half * 128:(half + 1) * 128].T

    def pack_lhsT(wm, nt):
        # wm [out, in]; returns [128, 2*nt*128]: [ci, (kh*nt+mt)*128+co]
        r = np.zeros((128, 2 * nt * 128), np.float32)
        for kh in range(2):
            for mt in range(nt):
                r[:, (kh * nt + mt) * 128:(kh * nt + mt) * 128 + 128] = \
                    wm[mt * 128:(mt + 1) * 128,
                       kh * 128:(kh + 1) * 128].T
        return r

    wq_p = pack_lhsT(f["w_q"], 1)
    wk_p = pack_lhsT(f["w_k"], 1)
    wv_p = pack_lhsT(f["w_v"], 2)
    wrow_p = pack_lhsT(f["w_row"], 2)
    wcol_p = pack_lhsT(f["w_col"], 2)
    wproj_p = pack_lhsT(f["w_proj"], 2)

    wpw_p = np.zeros((128, 4 * 256), np.float32)   # w_pw [256, 512]
    for kt in range(4):
        for mt in range(2):
            wpw_p[:, kt * 256 + mt * 128:kt * 256 + mt * 128 + 128] = \
                f["w_pw"][mt * 128:(mt + 1) * 128,
                          kt * 128:(kt + 1) * 128].T

    dwdg = np.zeros((128, 36 * 128), np.float32)   # w_dw [512,1,3,3]
    ii = np.arange(128)
    for t in range(4):
        for tap9 in range(9):
            ky, kx = divmod(tap9, 3)
            dwdg[ii, (t * 9 + tap9) * 128 + ii] = \
                f["w_dw"][t * 128 + ii, 0, ky, kx]

    post_p = np.zeros((16, 4 * 512), np.float32)
    for pidx, nm in enumerate(["pos_rowq", "pos_rowk", "pos_colq", "pos_colk"]):
        p = f[nm]                                   # [4, 128, 16]
        for i in range(4):
            post_p[:, (pidx * 4 + i) * 128:(pidx * 4 + i) * 128 + 128] = \
                p[i].T                              # [16, 128]

    biases = np.zeros((128, 20), np.float32)
    biases[:, B_CCAM + 0] = f["b_ccam_b"][:128]
    biases[:, B_CCAM + 1] = f["b_ccam_b"][128:]
    biases[:16, B_ENC] = f["b_enc"]
    biases[:, B_Q] = f["b_q"]
    biases[:, B_K] = f["b_k"]
    biases[:, B_V + 0] = f["b_v"][:128]
    biases[:, B_V + 1] = f["b_v"][128:]
    for t in range(4):
        biases[:, B_DW + t] = f["b_dw"][t * 128:(t + 1) * 128]
    biases[:, B_PW + 0] = f["b_pw"][:128]
    biases[:, B_PW + 1] = f["b_pw"][128:]
    biases[:, B_ROW + 0] = f["b_row"][:128]
    biases[:, B_ROW + 1] = f["b_row"][128:]
    biases[:, B_COL + 0] = f["b_col"][:128]
    biases[:, B_COL + 1] = f["b_col"][128:]
    biases[:, B_PROJ3 + 0] = f["b_proj"][:128] + 3.0
    biases[:, B_PROJ3 + 1] = f["b_proj"][128:] + 3.0

    return {
        "w3t": _bf(w3t), "wenc": _bf(wenc),
        "wq": _bf(wq_p), "wk": _bf(wk_p), "wv": _bf(wv_p),
        "wqs": _bf(wq_p / 32.0), "wks": _bf(wk_p / 32.0),
        "wvs": _bf(wv_p / 32.0),
        "dwd": _bf(dwdg), "wpw": _bf(wpw_p),
        "wrow": _bf(wrow_p), "wcol": _bf(wcol_p), "wproj": _bf(wproj_p),
        "post": _bf(post_p), "interpm": _bf(_interp_matrix()),
        "identb": _bf(np.eye(128)),
        "identf": np.eye(128, dtype=np.float32),
        "onesb": _bf(np.ones((128, 1))),
        "biases": np.ascontiguousarray(biases),
    }


def kernel(**inputs) -> np.ndarray:
    x = np.asarray(inputs["x"], np.float32)          # [8, 128, 128, 128]
    scale = float(np.asarray(inputs["scale_ccam"]).reshape(-1)[0])

    key = round(scale, 9)
    if key not in _CACHE:
        _CACHE[key] = build_graph(scale)
    nc = _CACHE[key]

    consts = prep_consts(inputs)
    in_maps = []
    for core in range(8):
        m = dict(consts)
        m["xb"] = np.ascontiguousarray(x[core].reshape(128, N))
        in_maps.append(m)

    res = run_bass_kernel_spmd(nc, in_maps, core_ids=list(range(8)))
    outs = [res.results[i]["out"].reshape(256, 128, 128) for i in range(8)]
    return np.stack(outs).astype(np.float32)


if __name__ == "__main__":
    rng = np.random.default_rng(0)
    demo = {"x": rng.standard_norm_((8, 128, 128, 128))} if False else None
    print("kernel module OK")


# revision 42
# speedup vs baseline: 1.0079x; 1.0013x over previous
"""Trainium2 Bass kernel for nn_Align_54279796687162 (sparse_attention).

Pure data parallel: one sample per NeuronCore (B=8 over 8 cores).
v3: all-bf16 datapath. cb/cf SBUF-resident; shunts computed in phase A via
linearity (shunt(xf) = shunt(cb) + attnT @ shunt(cf)); q/k/v computed
directly from cb/cf (q = wq@cb + (attnT@wq^T)^T@cf); single PE-bound
mega-loop: qkv -> DW (diag matmuls) -> PW -> xx -> proj -> out.
"""

import numpy as np
import ml_dtypes

import concourse.bass as bass
import concourse.mybir as mybir
import concourse.tile as tile
from concourse import bacc
from concourse.bass_utils import run_bass_kernel_spmd

BF = mybir.dt.bfloat16
F32 = mybir.dt.float32
AF = mybir.ActivationFunctionType
ALU = mybir.AluOpType
AX = mybir.AxisListType

H = W = 128
N = H * W            # 16384
BL = 512             # block size (4 rows * 128)
CH = 4               # chunks
SCALE = 0.25         # KD ** -0.5
PST = 132            # padded row stride for q/k/v (DW conv layout)
PSZ = PST * 130      # padded tensor size per partition
SST = 8 * PST        # q/k ring slot stride (8 rows: 4 data + 2+2 halo)

# bias column map in the packed [128, 20] f32 bias tile
B_CCAM, B_ENC, B_Q, B_K, B_V, B_DW, B_PW, B_ROW, B_COL, B_PROJ3 = (
    0, 2, 3, 4, 5, 7, 11, 13, 15, 17)

_CACHE = {}


def _ap(base, extra_off, free_dims):
    """Build an AP from a tile's base AP with custom free dims."""
    b = base[:]
    return bass.AP(b.tensor, b.offset + extra_off, [list(b.ap[0])] + free_dims)


def build_graph(scale_ccam: float):
    nc = bacc.Bacc(None, target_bir_lowering=False)

    xb = nc.dram_tensor("xb", [128, N], F32, kind="ExternalInput")
    w3t = nc.dram_tensor("w3t", [128, 9 * 256], BF, kind="ExternalInput")
    wenc = nc.dram_tensor("wenc", [128, 32], BF, kind="ExternalInput")
    wq = nc.dram_tensor("wq", [128, 256], BF, kind="ExternalInput")
    wk = nc.dram_tensor("wk", [128, 256], BF, kind="ExternalInput")
    wv = nc.dram_tensor("wv", [128, 512], BF, kind="ExternalInput")
    dwd = nc.dram_tensor("dwd", [128, 36 * 128], BF, kind="ExternalInput")
    wpw = nc.dram_tensor("wpw", [128, 4 * 256], BF, kind="ExternalInput")
    wqs = nc.dram_tensor("wqs", [128, 256], BF, kind="ExternalInput")
    wks = nc.dram_tensor("wks", [128, 256], BF, kind="ExternalInput")
    wvs = nc.dram_tensor("wvs", [128, 512], BF, kind="ExternalInput")
    wrow = nc.dram_tensor("wrow", [128, 512], BF, kind="ExternalInput")
    wcol = nc.dram_tensor("wcol", [128, 512], BF, kind="ExternalInput")
    wproj = nc.dram_tensor("wproj", [128, 512], BF, kind="ExternalInput")
    post = nc.dram_tensor("post", [16, 4 * 512], BF, kind="ExternalInput")
    interpm = nc.dram_tensor("interpm", [16, 128], BF, kind="ExternalInput")
    identb = nc.dram_tensor("identb", [128, 128], BF, kind="ExternalInput")
    identf = nc.dram_tensor("identf", [128, 128], F32, kind="ExternalInput")
    onesb = nc.dram_tensor("onesb", [128, 1], BF, kind="ExternalInput")
    biases = nc.dram_tensor("biases", [128, 20], F32, kind="ExternalInput")

    cb_dram = nc.dram_tensor("cb_dram", [2, 128, N], BF, kind="Internal")
    out = nc.dram_tensor("out", [256, N], F32, kind="ExternalOutput")

    with tile.TileContext(nc) as tc:
      with tc.tile_pool(name="cst", bufs=1) as cst:
        w3_s = cst.tile([128, 9 * 256], BF)
        idf_s = cst.tile([128, 128], F32)
        nc.sync.dma_start(w3_s[:], w3t[:])
        nc.sync.dma_start(idf_s[:], identf[:])
        wenc_s = cst.tile([128, 32], BF)
        wqb_s = cst.tile([128, 256], BF)
        wkb_s = cst.tile([128, 256], BF)
        wvb_s = cst.tile([128, 512], BF)
        wproj_s = cst.tile([128, 512], BF)
        idb_s = cst.tile([128, 128], BF)
        ones_s = cst.tile([128, 1], BF)
        bia_s = cst.tile([128, 20], F32)
        for t, d in [(wenc_s, wenc), (wqb_s, wq), (wkb_s, wk), (wvb_s, wv),
                     (wproj_s, wproj), (idb_s, identb),
                     (ones_s, onesb), (bia_s, biases)]:
            nc.sync.dma_start(t[:], d[:])

        wqs_s = cst.tile([128, 256], BF)
        wks_s = cst.tile([128, 256], BF)
        wvs_s = cst.tile([128, 512], BF)
        wrow_s = cst.tile([128, 512], BF)
        wcol_s = cst.tile([128, 512], BF)
        post_s = cst.tile([16, 4 * 512], BF)
        interp_s = cst.tile([16, 128], BF)
        for t, d in [(wqs_s, wqs), (wks_s, wks), (wvs_s, wvs),
                     (wrow_s, wrow), (wcol_s, wcol), (post_s, post),
                     (interp_s, interpm)]:
            nc.sync.dma_start(t[:], d[:])
        attnT_s = cst.tile([16, 256], BF)
        aq_s = cst.tile([16, 512], BF)       # (attnT @ w{q,k,v}^T) per target
        xfs_row = [cst.tile([128, 512], BF, tag=f"xfsr{h}", name=f"xfsr{h}") for h in range(2)]
        xfs_col = [cst.tile([128, 512], BF, tag=f"xfsc{h}", name=f"xfsc{h}") for h in range(2)]
        xproj = {(d_, t_): cst.tile([128, 512], BF, tag=f"xp{d_}{t_}", name=f"xp{d_}{t_}")
                 for d_ in range(2) for t_ in range(2)}

        with tc.tile_pool(name="pmid", bufs=1) as pmid:
          cf = pmid.tile([16, N], BF)

          # =========================================================
          # Phase A: conv3x3 -> cb ; cf ; shunts of cb/cf ; energy ;
          #          ccam softmax ; xfs assembly ; AQ
          # =========================================================
          with (
              tc.tile_pool(name="pa", bufs=1) as pa,
              tc.tile_pool(name="par", bufs=3) as par,
              tc.tile_pool(name="pamm", bufs=4, space="PSUM") as pamm,
              tc.tile_pool(name="patr", bufs=2, space="PSUM") as patr,
              tc.tile_pool(name="pae", bufs=2, space="PSUM") as pae,
          ):
            cb = [pa.tile([128, N], BF, tag=f"cb{h}", name=f"cb{h}")
                  for h in range(2)]
            xpad = pa.tile([128, 130 * 130], BF)

            # zero only the pad border of xpad; DMA x (f32->bf16) straight
            # into the interior, 32 rows at a time
            nc.vector.memset(_ap(xpad, 0, [[1, 130]]), 0.0)
            nc.vector.memset(_ap(xpad, 129 * 130, [[1, 130]]), 0.0)
            nc.vector.memset(_ap(xpad, 130, [[130, 128], [129, 2]]), 0.0)
            for rc in range(4):
                nc.gpsimd.dma_start(
                    _ap(xpad, 131 + rc * 32 * 130, [[130, 32], [1, 128]]),
                    _ap(xb, rc * 32 * 128, [[128, 32], [1, 128]]))

            scb_row = [pa.tile([128, 512], BF, tag=f"sbr{h}", name=f"sbr{h}")
                       for h in range(2)]
            scb_col = [pa.tile([128, 512], F32, tag=f"sbc{h}", name=f"sbc{h}")
                       for h in range(2)]
            scf_row = pa.tile([16, 512], BF)
            scf_col = pa.tile([16, 512], F32)
            scf_colb = pa.tile([16, 512], BF)

            # conv3x3: contiguous padded windows (junk cols stripped by
            # the ACT extraction copy), tap-major over 4-block psum groups
            cblk = [(r0, 3) for r0 in range(0, 126, 3)] + [(126, 2)]
            for half in range(2):
                for g0 in range(0, len(cblk), 2):
                    grp = cblk[g0:g0 + 2]
                    pss = [pamm.tile([128, BL], F32, tag="amm",
                                     name=f"cps{j}")
                           for j in range(len(grp))]
                    for t9 in range(9):
                        ky, kx = divmod(t9, 3)
                        for j, (r0, nr) in enumerate(grp):
                            rhs = _ap(xpad, (r0 + ky) * 130 + kx,
                                      [[1, nr * 130 - 2]])
                            nc.tensor.matmul(
                                _ap(pss[j], 0, [[1, nr * 130 - 2]]),
                                w3_s[:, t9 * 256 + half * 128:
                                     t9 * 256 + half * 128 + 128],
                                rhs, start=(t9 == 0), stop=(t9 == 8))
                    for j, (r0, nr) in enumerate(grp):
                        nc.scalar.activation(
                            cb[half][:, r0 * 128:(r0 + nr) * 128],
                            _ap(pss[j], 0, [[130, nr], [1, 128]]),
                            AF.Relu,
                            bias=bia_s[:, B_CCAM + half:B_CCAM + half + 1])
                nc.sync.dma_start(cb_dram[half, :, :], cb[half][:])

            # shunts of cb (row: mean over W%4 chunks; col: mean over H%4)
            # run on DVE/GpSimd while PE does enc + energy transposes
            for half in range(2):
                for b in range(32):
                    sl = slice(b * BL, (b + 1) * BL)
                    with nc.allow_low_precision(reason="bf16 shunt sums"):
                        src = _ap(cb[half], b * BL,
                                  [[1, 4], [128, 4], [4, 32]])
                        dst = _ap(scb_row[half], 4 * b,
                                  [[128, 4], [1, 4]])
                        nc.vector.tensor_reduce(dst, src, axis=AX.X,
                                                op=ALU.add)
                    ci = b // 8
                    part = par.tile([128, 128], F32, tag=f"cp{half}",
                                    name=f"cp{half}", bufs=2)
                    src = _ap(cb[half], b * BL, [[1, 128], [128, 4]])
                    nc.vector.tensor_reduce(part[:], src, axis=AX.X,
                                            op=ALU.add)
                    dstc = scb_col[half][:, ci * 128:(ci + 1) * 128]
                    if b % 8 == 0:
                        nc.gpsimd.tensor_copy(dstc, part[:])
                    else:
                        nc.gpsimd.tensor_tensor(dstc, dstc, part[:],
                                                ALU.add)

            # cf = relu(w_enc @ cb + b_enc)  -> [16, N]
            for bg in range(8):
                pss = [pamm.tile([16, BL], F32, tag="amm",
                                 name=f"fps{j}") for j in range(4)]
                for half in range(2):
                    for j in range(4):
                        b = bg * 4 + j
                        nc.tensor.matmul(
                            pss[j][:], wenc_s[:, half * 16:half * 16 + 16],
                            cb[half][:, b * BL:(b + 1) * BL],
                            start=(half == 0), stop=(half == 1))
                for j in range(4):
                    b = bg * 4 + j
                    nc.scalar.activation(
                        cf[:, b * BL:(b + 1) * BL], pss[j][:], AF.Relu,
                        bias=bia_s[:16, B_ENC:B_ENC + 1])

            # shunts of cf
            for b in range(32):
                with nc.allow_low_precision(reason="bf16 shunt sums"):
                    src = _ap(cf, b * BL, [[1, 4], [128, 4], [4, 32]])
                    dst = _ap(scf_row, 4 * b, [[128, 4], [1, 4]])
                    nc.vector.tensor_reduce(dst, src, axis=AX.X, op=ALU.add)
                ci = b // 8
                partf = par.tile([16, 128], F32, tag="cpf", bufs=2)
                src = _ap(cf, b * BL, [[1, 128], [128, 4]])
                nc.vector.tensor_reduce(partf[:], src, axis=AX.X, op=ALU.add)
                dstc = scf_col[:, ci * 128:(ci + 1) * 128]
                if b % 8 == 0:
                    nc.gpsimd.tensor_copy(dstc, partf[:])
                else:
                    nc.gpsimd.tensor_tensor(dstc, dstc, partf[:], ALU.add)

            # energy^T [16, 256] accumulated over 128 column-blocks.
            e_chain = [pae.tile([16, 256], F32, tag="ech", name=f"ech{c}")
                       for c in range(2)]
            for b in range(128):
                sl = slice(b * 128, (b + 1) * 128)
                tball = patr.tile([128, 272], BF, tag="tr")
                nc.tensor.matmul(tball[:, 0:128], cb[0][:, sl], idb_s[:],
                                 is_transpose=True, start=True, stop=False)
                nc.tensor.matmul(tball[:, 128:256], cb[1][:, sl], idb_s[:],
                                 is_transpose=True, start=False, stop=False)
                nc.tensor.matmul(tball[:, 256:272], cf[:, sl],
                                 idb_s[:16, :16],
                                 is_transpose=True, start=False, stop=True)
                bT = par.tile([128, 272], BF, tag="bT")
                nc.scalar.activation(bT[:], tball[:], AF.Copy)
                nc.tensor.matmul(e_chain[b % 2][:], bT[:, 256:272],
                                 bT[:, 0:256],
                                 start=(b < 2), stop=(b >= 126))

            # CCAM attention: attn = softmax(-energy) over K=16, store attn^T
            e_sb = pa.tile([16, 256], F32)
            e_tmp = pa.tile([16, 256], F32)
            nc.scalar.activation(e_tmp[:], e_chain[1][:], AF.Copy)
            nc.vector.tensor_tensor(e_sb[:], e_chain[0][:], e_tmp[:], ALU.add)
            at_half = []
            for half in range(2):
                tps = patr.tile([128, 16], F32, tag="tr")
                nc.tensor.transpose(
                    tps[:], e_sb[:, half * 128:(half + 1) * 128],
                    idf_s[:16, :16])
                e_c = par.tile([128, 16], F32, tag="ec")
                nc.vector.tensor_copy(e_c[:], tps[:])
                mn = par.tile([128, 1], F32, tag="mn")
                nc.vector.tensor_reduce(mn[:], e_c[:], axis=AX.X, op=ALU.min)
                ex = par.tile([128, 16], F32, tag="ex")
                nc.scalar.activation(ex[:], e_c[:], AF.Exp,
                                     bias=mn[:], scale=-1.0)
                sm = par.tile([128, 1], F32, tag="sm")
                nc.vector.tensor_reduce(sm[:], ex[:], axis=AX.X, op=ALU.add)
                rc = par.tile([128, 1], F32, tag="rc")
                nc.vector.reciprocal(rc[:], sm[:])
                at = par.tile([128, 16], BF, tag="at", bufs=2)
                nc.vector.tensor_scalar(at[:], ex[:], rc[:],
                                        float(scale_ccam), ALU.mult, ALU.mult)
                at_half.append(at)
                tat = patr.tile([16, 128], BF, tag="tr")
                nc.tensor.transpose(tat[:], at[:], idb_s[:])
                nc.vector.tensor_copy(
                    attnT_s[:, half * 128:(half + 1) * 128], tat[:])

            # AQ[m] = attnT @ w_m^T  -> [16, 4*128] (targets q,k,v0,v1)
            aq_ps = pae.tile([16, 512], F32, tag="ech", name="aq_ps")
            for m, (wt, mt) in enumerate([(wqb_s, 0), (wkb_s, 0),
                                          (wvb_s, 0), (wvb_s, 1)]):
                nt = 2 if wt is wvb_s else 1
                for kh in range(2):
                    nc.tensor.matmul(
                        aq_ps[:, m * 128:(m + 1) * 128], at_half[kh][:],
                        wt[:, (kh * nt + mt) * 128:(kh * nt + mt) * 128 + 128],
                        start=(kh == 0), stop=(kh == 1))
            nc.scalar.activation(aq_s[:], aq_ps[:], AF.Copy)

            # xfs = shunt(cb) + attnT @ shunt(cf)
            nc.vector.tensor_copy(scf_colb[:], scf_col[:])
            for half in range(2):
                ps = pamm.tile([128, BL], F32, tag="amm", name="xfr")
                nc.tensor.matmul(ps[:],
                                 attnT_s[:, half * 128:(half + 1) * 128],
                                 scf_row[:], start=True, stop=False)
                nc.tensor.matmul(ps[:], idb_s[:], scb_row[half][:],
                                 start=False, stop=True)
                nc.scalar.activation(xfs_row[half][:], ps[:], AF.Copy)
                ps2 = pamm.tile([128, BL], F32, tag="amm", name="xfc")
                nc.tensor.matmul(ps2[:],
                                 attnT_s[:, half * 128:(half + 1) * 128],
                                 scf_colb[:], start=True, stop=True)
                with nc.allow_low_precision(reason="bf16 xfs_col"):
                    nc.vector.tensor_tensor(xfs_col[half][:], ps2[:],
                                            scb_col[half][:], ALU.add)

          # =========================================================
          # Phase C1: axial attention (row: dir 0, col: dir 1)
          # =========================================================
          with (
              tc.tile_pool(name="pc", bufs=1) as pc,
              tc.tile_pool(name="pcr", bufs=3) as pcr,
              tc.tile_pool(name="pcm", bufs=2, space="PSUM") as pcm,
          ):
            pcmm = pcl = pcav = pcasm = pcm
            # --- interleave the two independent axial directions so one
            # direction's matmuls fill the other's pipeline latency ---
            st = {}
            for d_ in range(2):
                xfs = xfs_row if d_ == 0 else xfs_col
                qs_att = pc.tile([128, 512], BF, tag="qsa", name="qsa",
                                 bufs=2)
                ks_att = pc.tile([128, 512], BF, tag="ksa", name="ksa",
                                 bufs=2)
                vs_att = [pc.tile([128, 512], BF, tag=f"vsa{h}",
                                  name=f"vsa{h}", bufs=2)
                          for h in range(2)]
                st[d_] = dict(xfs=xfs, qs=qs_att, ks=ks_att, vs=vs_att)
                for (dst, wt, bc, nt, pidx) in [
                        ([qs_att], wqs_s, B_Q, 1, 2 * d_),
                        ([ks_att], wks_s, B_K, 1, 2 * d_ + 1),
                        (vs_att, wvs_s, B_V, 2, None)]:
                    for mt in range(nt):
                        ps = pcmm.tile([128, BL], F32, tag="cmm", bufs=2)
                        for kh in range(2):
                            nc.tensor.matmul(
                                ps[:],
                                wt[:, (kh * nt + mt) * 128:
                                   (kh * nt + mt) * 128 + 128],
                                xfs[kh][:], start=(kh == 0),
                                stop=(kh == 1 and pidx is None))
                        if pidx is not None:
                            for i in range(CH):
                                nc.tensor.matmul(
                                    ps[:, i * 128:(i + 1) * 128],
                                    post_s[:, (pidx * 4 + i) * 128:
                                           (pidx * 4 + i) * 128 + 128],
                                    interp_s[:], start=False, stop=(i == 3))
                        nc.scalar.activation(
                            dst[mt][:], ps[:], AF.Identity,
                            bias=bia_s[:, bc + mt:bc + mt + 1])

            for d_ in range(2):
                q_pack = pc.tile([128, 1024], BF, tag="qp", name="qp",
                                 bufs=2)
                k_pack = pc.tile([128, 1024], BF, tag="kp", name="kp",
                                 bufs=2)
                st[d_]["qp"] = q_pack
                st[d_]["kp"] = k_pack
                for g in range(8):
                    po, co = 32 * (g % 4), (g // 4) * 512
                    nc.sync.dma_start(
                        q_pack[po:po + 16, co:co + 512],
                        st[d_]["qs"][g * 16:(g + 1) * 16, :])
                    nc.sync.dma_start(
                        k_pack[po:po + 16, co:co + 512],
                        st[d_]["ks"][g * 16:(g + 1) * 16, :])

            for d_ in range(2):
                vt_s = pc.tile([128, 4, 256], BF, tag="vt", name="vt",
                               bufs=2)
                st[d_]["vt"] = vt_s
                for i in range(CH):
                    for hh in range(2):
                        tp = pcl.tile([128, 128], BF, tag="lps")
                        nc.tensor.transpose(
                            tp[:], st[d_]["vs"][hh][:, i * 128:(i + 1) * 128],
                            idb_s[:])
                        nc.scalar.activation(
                            vt_s[:, i, hh * 128:(hh + 1) * 128], tp[:],
                            AF.Copy)
                st[d_]["xpre"] = [
                    pc.tile([128, 512], BF, tag=f"xpre{t}",
                            name=f"xpre{t}", bufs=2) for t in range(2)]

            for i in range(CH):
                for th in range(2):
                    asm_ps = {d_: pcasm.tile([128, 128], BF, tag="asm",
                                             name=f"asm{d_}", bufs=2)
                              for d_ in range(2)}
                    for gg in range(4):
                        g = th * 4 + gg
                        po = 32 * (g % 4)
                        co = (g // 4) * 512
                        sl_gi = slice(co + i * 128, co + i * 128 + 128)
                        lps = {}
                        ets = {}
                        avs = {}
                        for d_ in range(2):
                            l_ps = pcl.tile([128, 128], F32, tag="lps",
                                            name=f"lps{d_}")
                            nc.tensor.matmul(l_ps[:],
                                             st[d_]["kp"][po:po + 16, sl_gi],
                                             st[d_]["qp"][po:po + 16, sl_gi],
                                             start=True, stop=True,
                                             tile_position=(po, 0))
                            lps[d_] = l_ps
                        for d_ in range(2):
                            e_t = pcr.tile([128, 128], BF, tag="et",
                                           name=f"et{d_}", bufs=4)
                            nc.scalar.activation(e_t[:], lps[d_][:], AF.Exp,
                                                 scale=SCALE)
                            ets[d_] = e_t
                        for d_ in range(2):
                            av_ps = pcav.tile([128, 33], F32, tag="av",
                                              name=f"av{d_}")
                            nc.tensor.matmul(
                                av_ps[:, 0:32], ets[d_][:],
                                st[d_]["vt"][:, i, g * 32:(g + 1) * 32],
                                start=True, stop=False)
                            nc.tensor.matmul(av_ps[:, 32:33], ets[d_][:],
                                             ones_s[:], start=False,
                                             stop=True)
                            avs[d_] = av_ps
                        xrns = {}
                        for d_ in range(2):
                            rcp = pcr.tile([128, 1], F32, tag="rcp",
                                           name=f"rcp{d_}", bufs=4)
                            nc.vector.reciprocal(rcp[:], avs[d_][:, 32:33])
                            xrn = pcr.tile([128, 32], BF, tag="xrn",
                                           name=f"xrn{d_}", bufs=4)
                            nc.vector.tensor_scalar(
                                xrn[:], avs[d_][:, 0:32], rcp[:], None,
                                ALU.mult)
                            xrns[d_] = xrn
                        for d_ in range(2):
                            nc.tensor.transpose(
                                asm_ps[d_][gg * 32:(gg + 1) * 32, :],
                                xrns[d_][:], idb_s[:],
                                tile_position=(0, gg * 32))
                    for d_ in range(2):
                        nc.scalar.activation(
                            st[d_]["xpre"][th][:, i * 128:(i + 1) * 128],
                            asm_ps[d_][:], AF.Relu)

            for d_ in range(2):
                wproj_d = wrow_s if d_ == 0 else wcol_s
                bcol = B_ROW if d_ == 0 else B_COL
                for mt in range(2):
                    ps = pcmm.tile([128, BL], F32, tag="cmm", bufs=2)
                    for kh in range(2):
                        nc.tensor.matmul(
                            ps[:],
                            wproj_d[:, (kh * 2 + mt) * 128:
                                    (kh * 2 + mt) * 128 + 128],
                            st[d_]["xpre"][kh][:], start=(kh == 0),
                            stop=(kh == 1))
                    nc.scalar.activation(
                        xproj[(d_, mt)][:], ps[:], AF.Identity,
                        bias=bia_s[:, bcol + mt:bcol + mt + 1])

          # =========================================================
          # Mega-loop: qkv[b] -> DW[b-1] -> PW[b-1] -> xx -> proj -> out
          # =========================================================
          with (
              tc.tile_pool(name="pf", bufs=1) as pf,
              tc.tile_pool(name="pfr", bufs=3) as pfr,
              tc.tile_pool(name="pbm", bufs=4, space="PSUM") as pbm,
              tc.tile_pool(name="pfd", bufs=3, space="PSUM") as pfd,
              tc.tile_pool(name="pfw", bufs=2, space="PSUM") as pfw,
          ):
            dwd_s = pf.tile([128, 36 * 128], BF)
            wpw_s = pf.tile([128, 4 * 256], BF)
            nc.sync.dma_start(dwd_s[:], dwd[:])
            nc.sync.dma_start(wpw_s[:], wpw[:])
            v_sb = [pf.tile([128, PSZ], BF, tag=f"v{h}", name=f"v{h}")
                    for h in range(2)]
            for t_ in v_sb:
                nc.gpsimd.memset(_ap(t_, 0, [[129 * PST, 2], [1, PST]]), 0.0)
                nc.gpsimd.memset(
                    _ap(t_, PST, [[PST, 128], [130, 2], [1, 2]]), 0.0)
            # q/k rings: 3 slots of 8 rows (2 halo + 4 data + 2 halo)
            qring = [pf.tile([128, SST], BF, name=f"qr{i}") for i in range(3)]
            kring = [pf.tile([128, SST], BF, name=f"kr{i}") for i in range(3)]
            for t_ in qring + kring:
                nc.gpsimd.memset(t_[:], 0.0)

            cbq = []
            def fetch_cb(bb):
                sl2 = slice(bb * BL, (bb + 1) * BL)
                t2 = [pfr.tile([128, BL], BF, tag=f"cbi{h}",
                               name=f"cbi{h}", bufs=4) for h in range(2)]
                nc.sync.dma_start(t2[0][:], cb_dram[0, :, sl2])
                nc.sync.dma_start(t2[1][:], cb_dram[1, :, sl2])
                cbq.append(t2)
            fetch_cb(0)
            fetch_cb(1)
            fetch_cb(2)
            for b in range(33):
              if b < 32:
                sl = slice(b * BL, (b + 1) * BL)
                if b + 3 < 32:
                    fetch_cb(b + 3)
                cbi = cbq[b]
                # q/k/v block b: 2 matmuls K=128 over cb halves + 1 K=16 (cf)
                for m, (wt, mt) in enumerate([(wqb_s, 0), (wkb_s, 0),
                                              (wvb_s, 0), (wvb_s, 1)]):
                    nt = 2 if wt is wvb_s else 1
                    ps = pbm.tile([128, BL], F32, tag="bmm",
                                  name=f"qps{m}")
                    for kh in range(2):
                        nc.tensor.matmul(
                            ps[:],
                            wt[:, (kh * nt + mt) * 128:
                               (kh * nt + mt) * 128 + 128],
                            cbi[kh][:], start=(kh == 0), stop=False)
                    nc.tensor.matmul(ps[:], aq_s[:, m * 128:(m + 1) * 128],
                                     cf[:, sl], start=False, stop=True)
                    bc = (B_Q, B_K, B_V, B_V + 1)[m]
                    if m < 2:
                        ring = qring if m == 0 else kring
                        pdst = _ap(ring[b % 3], 2 * PST + 2,
                                   [[PST, 4], [1, 128]])
                    else:
                        pdst = _ap(v_sb[m - 2], (4 * b + 1) * PST + 2,
                                   [[PST, 4], [1, 128]])
                    if m % 2 == 0:
                        nc.scalar.activation(
                            pdst, ps[:], AF.Identity,
                            bias=bia_s[:, bc:bc + 1])
                    else:
                        nc.vector.tensor_scalar(
                            pdst, ps[:], bia_s[:, bc:bc + 1],
                            None, ALU.add)
                # halo copies: slot b rows 0..1 <- slot b-1 rows 4..5;
                #              slot b-1 rows 6..7 <- slot b rows 2..3
                for ring in (qring, kring):
                    if b > 0:
                        nc.gpsimd.tensor_copy(
                            _ap(ring[b % 3], 0, [[1, 2 * PST]]),
                            _ap(ring[(b - 1) % 3], 4 * PST, [[1, 2 * PST]]))
                        nc.gpsimd.tensor_copy(
                            _ap(ring[(b - 1) % 3], 6 * PST, [[1, 2 * PST]]),
                            _ap(ring[b % 3], 2 * PST, [[1, 2 * PST]]))
                    else:
                        nc.gpsimd.memset(
                            _ap(ring[0], 0, [[1, 2 * PST]]), 0.0)
              if b == 0:
                  continue
              bp = b - 1          # tail block
              if bp == 31:
                  for ring in (qring, kring):
                      nc.gpsimd.memset(
                          _ap(ring[bp % 3], 6 * PST, [[1, 2 * PST]]), 0.0)

              # --- DW for block bp: 2-row groups j=0,1 ---
              dwg = [pfr.tile([128, 4 * 256], BF, tag=f"dwg{j}",
                              name=f"dwg{j}", bufs=3) for j in range(2)]
              for t in range(4):
                  dps = [pfd.tile([128, 260], F32, tag="dw",
                                  name=f"dps{j}", bufs=2) for j in range(2)]
                  for t9 in range(9):
                      ky, kx = divmod(t9, 3)
                      for j in range(2):
                          if t < 2:
                              src = (qring, kring)[t][bp % 3]
                              off = (1 + 2 * j + ky) * PST + kx + 1
                          else:
                              src = v_sb[t - 2]
                              off = (4 * bp + 2 * j + ky) * PST + kx + 1
                          nc.tensor.matmul(
                              _ap(dps[j], 0, [[1, 260]]),
                              dwd_s[:, (t * 9 + t9) * 128:
                                    (t * 9 + t9) * 128 + 128],
                              _ap(src, off, [[1, 260]]),
                              start=(t9 == 0), stop=(t9 == 8))
                  for j in range(2):
                      src = _ap(dps[j], 0, [[PST, 2], [1, 128]])
                      dst = _ap(dwg[j], t * 256, [[1, 256]])
                      if t % 2 == 0:
                          nc.scalar.activation(
                              dst, src, AF.Relu,
                              bias=bia_s[:, B_DW + t:B_DW + t + 1])
                      else:
                          nc.vector.tensor_scalar(
                              dst, src, bia_s[:, B_DW + t:B_DW + t + 1],
                              0.0, ALU.add, ALU.max)

              # --- PW: qo = w_pw @ dwg + b_pw ---
              pws = [pfw.tile([128, 2, 256], F32, tag="pw",
                              name=f"pws{j}") for j in range(2)]
              for mt in range(2):
                  for kt in range(4):
                      for j in range(2):
                          nc.tensor.matmul(
                              pws[j][:, mt, :],
                              wpw_s[:, kt * 256 + mt * 128:
                                    kt * 256 + mt * 128 + 128],
                              dwg[j][:, kt * 256:(kt + 1) * 256],
                              start=(kt == 0), stop=(kt == 3))
              qo_blk = [pfr.tile([128, BL], BF, tag=f"qo{mt}",
                                 name=f"qo{mt}", bufs=2) for mt in range(2)]
              for mt in range(2):
                  for j in range(2):
                      nc.scalar.activation(
                          qo_blk[mt][:, j * 256:(j + 1) * 256],
                          pws[j][:, mt, :], AF.Identity,
                          bias=bia_s[:, B_PW + mt:B_PW + mt + 1])

              # --- xx = relu(v + bcast(xrow) + bcast(xcol)) ---
              xxr = []
              for half in range(2):
                  xx = pfr.tile([128, BL], BF, tag=f"xx{half}",
                                name=f"xx{half}", bufs=2)
                  rap = _ap(xproj[(0, half)], bp * 16, [[1, 16], [0, 32]])
                  cap = _ap(xproj[(1, half)], (bp // 2) * 32,
                            [[0, 4], [0, 4], [1, 32]])
                  nc.vector.tensor_tensor(xx[:], rap, cap, ALU.add)
                  vap = _ap(v_sb[half], (4 * bp + 1) * PST + 2,
                            [[PST, 4], [1, 128]])
                  nc.vector.tensor_tensor(xx[:], xx[:], vap, ALU.add)
                  nc.vector.tensor_scalar(xx[:], xx[:], 0.0, None,
                                          ALU.max)
                  xxr.append(xx)

              # --- proj + hsig + multiply qo, DMA out ---
              for mt in range(2):
                  ps = pbm.tile([128, BL], F32, tag="bmm", name="prj")
                  for kh in range(2):
                      nc.tensor.matmul(
                          ps[:],
                          wproj_s[:, (kh * 2 + mt) * 128:
                                  (kh * 2 + mt) * 128 + 128],
                          xxr[kh][:], start=(kh == 0), stop=(kh == 1))
                  hs = pfr.tile([128, BL], BF, tag="hs", bufs=2)
                  nc.scalar.activation(
                      hs[:], ps[:], AF.Relu,
                      bias=bia_s[:, B_PROJ3 + mt:B_PROJ3 + mt + 1])
                  att_t = pfr.tile([128, BL], BF, tag="att", bufs=2)
                  nc.vector.tensor_scalar(att_t[:], hs[:], 6.0,
                                          1.0 / 6.0, ALU.min, ALU.mult)
                  ob = pfr.tile([128, BL], BF, tag="ob", bufs=2)
                  nc.vector.tensor_tensor(ob[:], att_t[:], qo_blk[mt][:],
                                          ALU.mult)
                  slp = slice(bp * BL, (bp + 1) * BL)
                  nc.gpsimd.dma_start(out[mt * 128:(mt + 1) * 128, slp],
                                      ob[:])

    nc.compile()
    return nc


def _interp_matrix():
    s, n = 16, 128
    src = np.clip((np.arange(n) + 0.5) * (s / n) - 0.5, 0.0, s - 1.0)
    i0 = np.floor(src).astype(np.int64)
    i1 = np.minimum(i0 + 1, s - 1)
    w = src - i0
    M = np.zeros((s, n), np.float64)
    np.add.at(M, (i0, np.arange(n)), 1.0 - w)
    np.add.at(M, (i1, np.arange(n)), w)
    return M


def _bf(x):
    return np.ascontiguousarray(np.asarray(x, np.float32).astype(
        ml_dtypes.bfloat16))


def prep_consts(inputs):
    """Host-side layout prep of all weight tensors (shared across cores)."""
    f = {k: np.asarray(v, np.float32) for k, v in inputs.items()}

    w3 = f["w_ccam_b"]                      # [256, 128, 3, 3]
    w3t = np.zeros((128, 9 * 256), np.float32)
    for ky in range(3):
        for kx in range(3):
            t9 = ky * 3 + kx
            w3t[:, t9 * 256:(t9 + 1) * 256] = w3[:, :, ky, kx].T
    wenc = np.zeros((128, 32), np.float32)  # w_enc [16, 256]
    for half in range(2):
        wenc[:, half * 16:(half + 1) * 16] = \
            f["w_enc"][:, half * 128:(half + 1) * 128].T

    def pack_lhsT(wm, nt):
        # wm [out, in]; returns [128, 2*nt*128]: [ci, (kh*nt+mt)*128+co]
        r = np.zeros((128, 2 * nt * 128), np.float32)
        for kh in range(2):
            for mt in range(nt):
                r[:, (kh * nt + mt) * 128:(kh * nt + mt) * 128 + 128] = \
                    wm[mt * 128:(mt + 1) * 128,
                       kh * 128:(kh + 1) * 128].T
        return r

    wq_p = pack_lhsT(f["w_q"], 1)
    wk_p = pack_lhsT(f["w_k"], 1)
    wv_p = pack_lhsT(f["w_v"], 2)
    wrow_p = pack_lhsT(f["w_row"], 2)
    wcol_p = pack_lhsT(f["w_col"], 2)
    wproj_p = pack_lhsT(f["w_proj"], 2)

    wpw_p = np.zeros((128, 4 * 256), np.float32)   # w_pw [256, 512]
    for kt in range(4):
        for mt in range(2):
            wpw_p[:, kt * 256 + mt * 128:kt * 256 + mt * 128 + 128] = \
                f["w_pw"][mt * 128:(mt + 1) * 128,
                          kt * 128:(kt + 1) * 128].T

    dwdg = np.zeros((128, 36 * 128), np.float32)   # w_dw [512,1,3,3]
    ii = np.arange(128)
    for t in range(4):
        for tap9 in range(9):
            ky, kx = divmod(tap9, 3)
            dwdg[ii, (t * 9 + tap9) * 128 + ii] = \
                f["w_dw"][t * 128 + ii, 0, ky, kx]

    post_p = np.zeros((16, 4 * 512), np.float32)
    for pidx, nm in enumerate(["pos_rowq", "pos_rowk", "pos_colq", "pos_colk"]):
        p = f[nm]                                   # [4, 128, 16]
        for i in range(4):
            post_p[:, (pidx * 4 + i) * 128:(pidx * 4 + i) * 128 + 128] = \
                p[i].T                              # [16, 128]

    biases = np.zeros((128, 20), np.float32)
    biases[:, B_CCAM + 0] = f["b_ccam_b"][:128]
    biases[:, B_CCAM + 1] = f["b_ccam_b"][128:]
    biases[:16, B_ENC] = f["b_enc"]
    biases[:, B_Q] = f["b_q"]
    biases[:, B_K] = f["b_k"]
    biases[:, B_V + 0] = f["b_v"][:128]
    biases[:, B_V + 1] = f["b_v"][128:]
    for t in range(4):
        biases[:, B_DW + t] = f["b_dw"][t * 128:(t + 1) * 128]
    biases[:, B_PW + 0] = f["b_pw"][:128]
    biases[:, B_PW + 1] = f["b_pw"][128:]
    biases[:, B_ROW + 0] = f["b_row"][:128]
    biases[:, B_ROW + 1] = f["b_row"][128:]
    biases[:, B_COL + 0] = f["b_col"][:128]
    biases[:, B_COL + 1] = f["b_col"][128:]
    biases[:, B_PROJ3 + 0] = f["b_proj"][:128] + 3.0
    biases[:, B_PROJ3 + 1] = f["b_proj"][128:] + 3.0

    return {
        "w3t": _bf(w3t), "wenc": _bf(wenc),
        "wq": _bf(wq_p), "wk": _bf(wk_p), "wv": _bf(wv_p),
        "wqs": _bf(wq_p / 32.0), "wks": _bf(wk_p / 32.0),
        "wvs": _bf(wv_p / 32.0),
        "dwd": _bf(dwdg), "wpw": _bf(wpw_p),
        "wrow": _bf(wrow_p), "wcol": _bf(wcol_p), "wproj": _bf(wproj_p),
        "post": _bf(post_p), "interpm": _bf(_interp_matrix()),
        "identb": _bf(np.eye(128)),
        "identf": np.eye(128, dtype=np.float32),
        "onesb": _bf(np.ones((128, 1))),
        "biases": np.ascontiguousarray(biases),
    }


def kernel(**inputs) -> np.ndarray:
    x = np.asarray(inputs["x"], np.float32)          # [8, 128, 128, 128]
    scale = float(np.asarray(inputs["scale_ccam"]).reshape(-1)[0])

    key = round(scale, 9)
    if key not in _CACHE:
        _CACHE[key] = build_graph(scale)
    nc = _CACHE[key]

    consts = prep_consts(inputs)
    in_maps = []
    for core in range(8):
        m = dict(consts)
        m["xb"] = np.ascontiguousarray(x[core].reshape(128, N))
        in_maps.append(m)

    res = run_bass_kernel_spmd(nc, in_maps, core_ids=list(range(8)))
    outs = [res.results[i]["out"].reshape(256, 128, 128) for i in range(8)]
    return np.stack(outs).astype(np.float32)


if __name__ == "__main__":
    rng = np.random.default_rng(0)
    demo = {"x": rng.standard_norm_((8, 128, 128, 128))} if False else None
    print("kernel module OK")


# revision 43
# speedup vs baseline: 1.0104x; 1.0024x over previous
"""Trainium2 Bass kernel for nn_Align_54279796687162 (sparse_attention).

Pure data parallel: one sample per NeuronCore (B=8 over 8 cores).
v3: all-bf16 datapath. cb/cf SBUF-resident; shunts computed in phase A via
linearity (shunt(xf) = shunt(cb) + attnT @ shunt(cf)); q/k/v computed
directly from cb/cf (q = wq@cb + (attnT@wq^T)^T@cf); single PE-bound
mega-loop: qkv -> DW (diag matmuls) -> PW -> xx -> proj -> out.
"""

import numpy as np
import ml_dtypes

import concourse.bass as bass
import concourse.mybir as mybir
import concourse.tile as tile
from concourse import bacc
from concourse.bass_utils import run_bass_kernel_spmd

BF = mybir.dt.bfloat16
F32 = mybir.dt.float32
AF = mybir.ActivationFunctionType
ALU = mybir.AluOpType
AX = mybir.AxisListType

H = W = 128
N = H * W            # 16384
BL = 512             # block size (4 rows * 128)
CH = 4               # chunks
SCALE = 0.25         # KD ** -0.5
PST = 132            # padded row stride for q/k/v (DW conv layout)
PSZ = PST * 130      # padded tensor size per partition
SST = 8 * PST        # q/k ring slot stride (8 rows: 4 data + 2+2 halo)

# bias column map in the packed [128, 20] f32 bias tile
B_CCAM, B_ENC, B_Q, B_K, B_V, B_DW, B_PW, B_ROW, B_COL, B_PROJ3 = (
    0, 2, 3, 4, 5, 7, 11, 13, 15, 17)

_CACHE = {}


def _ap(base, extra_off, free_dims):
    """Build an AP from a tile's base AP with custom free dims."""
    b = base[:]
    return bass.AP(b.tensor, b.offset + extra_off, [list(b.ap[0])] + free_dims)


def build_graph(scale_ccam: float):
    nc = bacc.Bacc(None, target_bir_lowering=False)

    xb = nc.dram_tensor("xb", [128, N], F32, kind="ExternalInput")
    w3t = nc.dram_tensor("w3t", [128, 9 * 256], BF, kind="ExternalInput")
    wenc = nc.dram_tensor("wenc", [128, 32], BF, kind="ExternalInput")
    wq = nc.dram_tensor("wq", [128, 256], BF, kind="ExternalInput")
    wk = nc.dram_tensor("wk", [128, 256], BF, kind="ExternalInput")
    wv = nc.dram_tensor("wv", [128, 512], BF, kind="ExternalInput")
    dwd = nc.dram_tensor("dwd", [128, 36 * 128], BF, kind="ExternalInput")
    wpw = nc.dram_tensor("wpw", [128, 4 * 256], BF, kind="ExternalInput")
    wqs = nc.dram_tensor("wqs", [128, 256], BF, kind="ExternalInput")
    wks = nc.dram_tensor("wks", [128, 256], BF, kind="ExternalInput")
    wvs = nc.dram_tensor("wvs", [128, 512], BF, kind="ExternalInput")
    wrow = nc.dram_tensor("wrow", [128, 512], BF, kind="ExternalInput")
    wcol = nc.dram_tensor("wcol", [128, 512], BF, kind="ExternalInput")
    wproj = nc.dram_tensor("wproj", [128, 512], BF, kind="ExternalInput")
    post = nc.dram_tensor("post", [16, 4 * 512], BF, kind="ExternalInput")
    interpm = nc.dram_tensor("interpm", [16, 128], BF, kind="ExternalInput")
    identb = nc.dram_tensor("identb", [128, 128], BF, kind="ExternalInput")
    identf = nc.dram_tensor("identf", [128, 128], F32, kind="ExternalInput")
    onesb = nc.dram_tensor("onesb", [128, 1], BF, kind="ExternalInput")
    biases = nc.dram_tensor("biases", [128, 20], F32, kind="ExternalInput")

    cb_dram = nc.dram_tensor("cb_dram", [2, 128, N], BF, kind="Internal")
    out = nc.dram_tensor("out", [256, N], F32, kind="ExternalOutput")

    with tile.TileContext(nc) as tc:
      with tc.tile_pool(name="cst", bufs=1) as cst:
        w3_s = cst.tile([128, 9 * 256], BF)
        idf_s = cst.tile([128, 128], F32)
        nc.sync.dma_start(w3_s[:], w3t[:])
        nc.sync.dma_start(idf_s[:], identf[:])
        wenc_s = cst.tile([128, 32], BF)
        wqb_s = cst.tile([128, 256], BF)
        wkb_s = cst.tile([128, 256], BF)
        wvb_s = cst.tile([128, 512], BF)
        wproj_s = cst.tile([128, 512], BF)
        idb_s = cst.tile([128, 128], BF)
        ones_s = cst.tile([128, 1], BF)
        bia_s = cst.tile([128, 20], F32)
        for t, d in [(wenc_s, wenc), (wqb_s, wq), (wkb_s, wk), (wvb_s, wv),
                     (wproj_s, wproj), (idb_s, identb),
                     (ones_s, onesb), (bia_s, biases)]:
            nc.sync.dma_start(t[:], d[:])

        wqs_s = cst.tile([128, 256], BF)
        wks_s = cst.tile([128, 256], BF)
        wvs_s = cst.tile([128, 512], BF)
        wrow_s = cst.tile([128, 512], BF)
        wcol_s = cst.tile([128, 512], BF)
        post_s = cst.tile([16, 4 * 512], BF)
        interp_s = cst.tile([16, 128], BF)
        for t, d in [(wqs_s, wqs), (wks_s, wks), (wvs_s, wvs),
                     (wrow_s, wrow), (wcol_s, wcol), (post_s, post),
                     (interp_s, interpm)]:
            nc.sync.dma_start(t[:], d[:])
        attnT_s = cst.tile([16, 256], BF)
        aq_s = cst.tile([16, 512], BF)       # (attnT @ w{q,k,v}^T) per target
        xfs_row = [cst.tile([128, 512], BF, tag=f"xfsr{h}", name=f"xfsr{h}") for h in range(2)]
        xfs_col = [cst.tile([128, 512], BF, tag=f"xfsc{h}", name=f"xfsc{h}") for h in range(2)]
        xproj = {(d_, t_): cst.tile([128, 512], BF, tag=f"xp{d_}{t_}", name=f"xp{d_}{t_}")
                 for d_ in range(2) for t_ in range(2)}

        with tc.tile_pool(name="pmid", bufs=1) as pmid:
          cf = pmid.tile([16, N], BF)

          # =========================================================
          # Phase A: conv3x3 -> cb ; cf ; shunts of cb/cf ; energy ;
          #          ccam softmax ; xfs assembly ; AQ
          # =========================================================
          with (
              tc.tile_pool(name="pa", bufs=1) as pa,
              tc.tile_pool(name="par", bufs=3) as par,
              tc.tile_pool(name="pamm", bufs=4, space="PSUM") as pamm,
              tc.tile_pool(name="patr", bufs=2, space="PSUM") as patr,
              tc.tile_pool(name="pae", bufs=2, space="PSUM") as pae,
          ):
            cb = [pa.tile([128, N], BF, tag=f"cb{h}", name=f"cb{h}")
                  for h in range(2)]
            xpad = pa.tile([128, 130 * 130], BF)

            # zero only the pad border of xpad; DMA x (f32->bf16) straight
            # into the interior, 32 rows at a time
            nc.vector.memset(_ap(xpad, 0, [[1, 130]]), 0.0)
            nc.vector.memset(_ap(xpad, 129 * 130, [[1, 130]]), 0.0)
            nc.vector.memset(_ap(xpad, 130, [[130, 128], [129, 2]]), 0.0)
            for rc in range(4):
                nc.gpsimd.dma_start(
                    _ap(xpad, 131 + rc * 32 * 130, [[130, 32], [1, 128]]),
                    _ap(xb, rc * 32 * 128, [[128, 32], [1, 128]]))

            scb_row = [pa.tile([128, 512], BF, tag=f"sbr{h}", name=f"sbr{h}")
                       for h in range(2)]
            scb_col = [pa.tile([128, 512], F32, tag=f"sbc{h}", name=f"sbc{h}")
                       for h in range(2)]
            scf_row = pa.tile([16, 512], BF)
            scf_col = pa.tile([16, 512], F32)
            scf_colb = pa.tile([16, 512], BF)

            # conv3x3: contiguous padded windows (junk cols stripped by
            # the ACT extraction copy), tap-major over 4-block psum groups
            cblk = [(r0, 3) for r0 in range(0, 126, 3)] + [(126, 2)]
            for half in range(2):
                for g0 in range(0, len(cblk), 2):
                    grp = cblk[g0:g0 + 2]
                    pss = [pamm.tile([128, BL], F32, tag="amm",
                                     name=f"cps{j}")
                           for j in range(len(grp))]
                    for t9 in range(9):
                        ky, kx = divmod(t9, 3)
                        for j, (r0, nr) in enumerate(grp):
                            rhs = _ap(xpad, (r0 + ky) * 130 + kx,
                                      [[1, nr * 130 - 2]])
                            nc.tensor.matmul(
                                _ap(pss[j], 0, [[1, nr * 130 - 2]]),
                                w3_s[:, t9 * 256 + half * 128:
                                     t9 * 256 + half * 128 + 128],
                                rhs, start=(t9 == 0), stop=(t9 == 8))
                    for j, (r0, nr) in enumerate(grp):
                        nc.scalar.activation(
                            cb[half][:, r0 * 128:(r0 + nr) * 128],
                            _ap(pss[j], 0, [[130, nr], [1, 128]]),
                            AF.Relu,
                            bias=bia_s[:, B_CCAM + half:B_CCAM + half + 1])
                nc.sync.dma_start(cb_dram[half, :, :], cb[half][:])

            # shunts of cb (row: mean over W%4 chunks; col: mean over H%4)
            # run on DVE/GpSimd while PE does enc + energy transposes
            for half in range(2):
                for b in range(32):
                    sl = slice(b * BL, (b + 1) * BL)
                    with nc.allow_low_precision(reason="bf16 shunt sums"):
                        src = _ap(cb[half], b * BL,
                                  [[1, 4], [128, 4], [4, 32]])
                        dst = _ap(scb_row[half], 4 * b,
                                  [[128, 4], [1, 4]])
                        nc.vector.tensor_reduce(dst, src, axis=AX.X,
                                                op=ALU.add)
                    ci = b // 8
                    part = par.tile([128, 128], F32, tag=f"cp{half}",
                                    name=f"cp{half}", bufs=2)
                    src = _ap(cb[half], b * BL, [[1, 128], [128, 4]])
                    nc.vector.tensor_reduce(part[:], src, axis=AX.X,
                                            op=ALU.add)
                    dstc = scb_col[half][:, ci * 128:(ci + 1) * 128]
                    if b % 8 == 0:
                        nc.gpsimd.tensor_copy(dstc, part[:])
                    else:
                        nc.gpsimd.tensor_tensor(dstc, dstc, part[:],
                                                ALU.add)

            # cf = relu(w_enc @ cb + b_enc)  -> [16, N]
            for bg in range(8):
                pss = [pamm.tile([16, BL], F32, tag="amm",
                                 name=f"fps{j}") for j in range(4)]
                for half in range(2):
                    for j in range(4):
                        b = bg * 4 + j
                        nc.tensor.matmul(
                            pss[j][:], wenc_s[:, half * 16:half * 16 + 16],
                            cb[half][:, b * BL:(b + 1) * BL],
                            start=(half == 0), stop=(half == 1))
                for j in range(4):
                    b = bg * 4 + j
                    nc.scalar.activation(
                        cf[:, b * BL:(b + 1) * BL], pss[j][:], AF.Relu,
                        bias=bia_s[:16, B_ENC:B_ENC + 1])

            # shunts of cf
            for b in range(32):
                with nc.allow_low_precision(reason="bf16 shunt sums"):
                    src = _ap(cf, b * BL, [[1, 4], [128, 4], [4, 32]])
                    dst = _ap(scf_row, 4 * b, [[128, 4], [1, 4]])
                    nc.vector.tensor_reduce(dst, src, axis=AX.X, op=ALU.add)
                ci = b // 8
                partf = par.tile([16, 128], F32, tag="cpf", bufs=2)
                src = _ap(cf, b * BL, [[1, 128], [128, 4]])
                nc.vector.tensor_reduce(partf[:], src, axis=AX.X, op=ALU.add)
                dstc = scf_col[:, ci * 128:(ci + 1) * 128]
                if b % 8 == 0:
                    nc.gpsimd.tensor_copy(dstc, partf[:])
                else:
                    nc.gpsimd.tensor_tensor(dstc, dstc, partf[:], ALU.add)

            # energy^T [16, 256] accumulated over 128 column-blocks.
            e_chain = [pae.tile([16, 256], F32, tag="ech", name=f"ech{c}")
                       for c in range(2)]
            for b in range(128):
                sl = slice(b * 128, (b + 1) * 128)
                tball = patr.tile([128, 272], BF, tag="tr")
                nc.tensor.matmul(tball[:, 0:128], cb[0][:, sl], idb_s[:],
                                 is_transpose=True, start=True, stop=False)
                nc.tensor.matmul(tball[:, 128:256], cb[1][:, sl], idb_s[:],
                                 is_transpose=True, start=False, stop=False)
                nc.tensor.matmul(tball[:, 256:272], cf[:, sl],
                                 idb_s[:16, :16],
                                 is_transpose=True, start=False, stop=True)
                bT = par.tile([128, 272], BF, tag="bT")
                nc.scalar.activation(bT[:], tball[:], AF.Copy)
                nc.tensor.matmul(e_chain[b % 2][:], bT[:, 256:272],
                                 bT[:, 0:256],
                                 start=(b < 2), stop=(b >= 126))

            # CCAM attention: attn = softmax(-energy) over K=16, store attn^T
            e_sb = pa.tile([16, 256], F32)
            e_tmp = pa.tile([16, 256], F32)
            nc.scalar.activation(e_tmp[:], e_chain[1][:], AF.Copy)
            nc.vector.tensor_tensor(e_sb[:], e_chain[0][:], e_tmp[:], ALU.add)
            at_half = []
            for half in range(2):
                tps = patr.tile([128, 16], F32, tag="tr")
                nc.tensor.transpose(
                    tps[:], e_sb[:, half * 128:(half + 1) * 128],
                    idf_s[:16, :16])
                e_c = par.tile([128, 16], F32, tag="ec")
                nc.vector.tensor_copy(e_c[:], tps[:])
                mn = par.tile([128, 1], F32, tag="mn")
                nc.vector.tensor_reduce(mn[:], e_c[:], axis=AX.X, op=ALU.min)
                ex = par.tile([128, 16], F32, tag="ex")
                nc.scalar.activation(ex[:], e_c[:], AF.Exp,
                                     bias=mn[:], scale=-1.0)
                sm = par.tile([128, 1], F32, tag="sm")
                nc.vector.tensor_reduce(sm[:], ex[:], axis=AX.X, op=ALU.add)
                rc = par.tile([128, 1], F32, tag="rc")
                nc.vector.reciprocal(rc[:], sm[:])
                at = par.tile([128, 16], BF, tag="at", bufs=2)
                nc.vector.tensor_scalar(at[:], ex[:], rc[:],
                                        float(scale_ccam), ALU.mult, ALU.mult)
                at_half.append(at)
                tat = patr.tile([16, 128], BF, tag="tr")
                nc.tensor.transpose(tat[:], at[:], idb_s[:])
                nc.vector.tensor_copy(
                    attnT_s[:, half * 128:(half + 1) * 128], tat[:])

            # AQ[m] = attnT @ w_m^T  -> [16, 4*128] (targets q,k,v0,v1)
            aq_ps = pae.tile([16, 512], F32, tag="ech", name="aq_ps")
            for m, (wt, mt) in enumerate([(wqb_s, 0), (wkb_s, 0),
                                          (wvb_s, 0), (wvb_s, 1)]):
                nt = 2 if wt is wvb_s else 1
                for kh in range(2):
                    nc.tensor.matmul(
                        aq_ps[:, m * 128:(m + 1) * 128], at_half[kh][:],
                        wt[:, (kh * nt + mt) * 128:(kh * nt + mt) * 128 + 128],
                        start=(kh == 0), stop=(kh == 1))
            nc.scalar.activation(aq_s[:], aq_ps[:], AF.Copy)

            # xfs = shunt(cb) + attnT @ shunt(cf)
            nc.vector.tensor_copy(scf_colb[:], scf_col[:])
            for half in range(2):
                ps = pamm.tile([128, BL], F32, tag="amm", name="xfr")
                nc.tensor.matmul(ps[:],
                                 attnT_s[:, half * 128:(half + 1) * 128],
                                 scf_row[:], start=True, stop=False)
                nc.tensor.matmul(ps[:], idb_s[:], scb_row[half][:],
                                 start=False, stop=True)
                nc.scalar.activation(xfs_row[half][:], ps[:], AF.Copy)
                ps2 = pamm.tile([128, BL], F32, tag="amm", name="xfc")
                nc.tensor.matmul(ps2[:],
                                 attnT_s[:, half * 128:(half + 1) * 128],
                                 scf_colb[:], start=True, stop=True)
                with nc.allow_low_precision(reason="bf16 xfs_col"):
                    nc.vector.tensor_tensor(xfs_col[half][:], ps2[:],
                                            scb_col[half][:], ALU.add)

          # =========================================================
          # Phase C1: axial attention (row: dir 0, col: dir 1)
          # =========================================================
          with (
              tc.tile_pool(name="pc", bufs=1) as pc,
              tc.tile_pool(name="pcr", bufs=3) as pcr,
              tc.tile_pool(name="pcm", bufs=2, space="PSUM") as pcm,
          ):
            pcmm = pcl = pcav = pcasm = pcm
            # --- interleave the two independent axial directions so one
            # direction's matmuls fill the other's pipeline latency ---
            st = {}
            for d_ in range(2):
                xfs = xfs_row if d_ == 0 else xfs_col
                qs_att = pc.tile([128, 512], BF, tag="qsa", name="qsa",
                                 bufs=2)
                ks_att = pc.tile([128, 512], BF, tag="ksa", name="ksa",
                                 bufs=2)
                vs_att = [pc.tile([128, 512], BF, tag=f"vsa{h}",
                                  name=f"vsa{h}", bufs=2)
                          for h in range(2)]
                st[d_] = dict(xfs=xfs, qs=qs_att, ks=ks_att, vs=vs_att)
                for (dst, wt, bc, nt, pidx) in [
                        ([qs_att], wqs_s, B_Q, 1, 2 * d_),
                        ([ks_att], wks_s, B_K, 1, 2 * d_ + 1),
                        (vs_att, wvs_s, B_V, 2, None)]:
                    for mt in range(nt):
                        ps = pcmm.tile([128, BL], F32, tag="cmm", bufs=2)
                        for kh in range(2):
                            nc.tensor.matmul(
                                ps[:],
                                wt[:, (kh * nt + mt) * 128:
                                   (kh * nt + mt) * 128 + 128],
                                xfs[kh][:], start=(kh == 0),
                                stop=(kh == 1 and pidx is None))
                        if pidx is not None:
                            for i in range(CH):
                                nc.tensor.matmul(
                                    ps[:, i * 128:(i + 1) * 128],
                                    post_s[:, (pidx * 4 + i) * 128:
                                           (pidx * 4 + i) * 128 + 128],
                                    interp_s[:], start=False, stop=(i == 3))
                        nc.scalar.activation(
                            dst[mt][:], ps[:], AF.Identity,
                            bias=bia_s[:, bc + mt:bc + mt + 1])

            for d_ in range(2):
                q_pack = pc.tile([128, 1024], BF, tag="qp", name="qp",
                                 bufs=2)
                k_pack = pc.tile([128, 1024], BF, tag="kp", name="kp",
                                 bufs=2)
                st[d_]["qp"] = q_pack
                st[d_]["kp"] = k_pack
                for g in range(8):
                    po, co = 32 * (g % 4), (g // 4) * 512
                    nc.sync.dma_start(
                        q_pack[po:po + 16, co:co + 512],
                        st[d_]["qs"][g * 16:(g + 1) * 16, :])
                    nc.sync.dma_start(
                        k_pack[po:po + 16, co:co + 512],
                        st[d_]["ks"][g * 16:(g + 1) * 16, :])

            for d_ in range(2):
                vt_s = pc.tile([128, 4, 256], BF, tag="vt", name="vt",
                               bufs=2)
                st[d_]["vt"] = vt_s
                for i in range(CH):
                    for hh in range(2):
                        tp = pcl.tile([128, 128], BF, tag="lps")
                        nc.tensor.transpose(
                            tp[:], st[d_]["vs"][hh][:, i * 128:(i + 1) * 128],
                            idb_s[:])
                        nc.scalar.activation(
                            vt_s[:, i, hh * 128:(hh + 1) * 128], tp[:],
                            AF.Copy)
                st[d_]["xpre"] = [
                    pc.tile([128, 512], BF, tag=f"xpre{t}",
                            name=f"xpre{t}", bufs=2) for t in range(2)]

            for i in range(CH):
                for th in range(2):
                    asm_ps = {d_: pcasm.tile([128, 128], BF, tag="asm",
                                             name=f"asm{d_}", bufs=2)
                              for d_ in range(2)}
                    for gg in range(4):
                        g = th * 4 + gg
                        po = 32 * (g % 4)
                        co = (g // 4) * 512
                        sl_gi = slice(co + i * 128, co + i * 128 + 128)
                        lps = {}
                        ets = {}
                        avs = {}
                        for d_ in range(2):
                            l_ps = pcl.tile([128, 128], F32, tag="lps",
                                            name=f"lps{d_}")
                            nc.tensor.matmul(l_ps[:],
                                             st[d_]["kp"][po:po + 16, sl_gi],
                                             st[d_]["qp"][po:po + 16, sl_gi],
                                             start=True, stop=True,
                                             tile_position=(po, 0))
                            lps[d_] = l_ps
                        for d_ in range(2):
                            e_t = pcr.tile([128, 128], BF, tag="et",
                                           name=f"et{d_}", bufs=4)
                            nc.scalar.activation(e_t[:], lps[d_][:], AF.Exp,
                                                 scale=SCALE)
                            ets[d_] = e_t
                        for d_ in range(2):
                            av_ps = pcav.tile([128, 33], F32, tag="av",
                                              name=f"av{d_}")
                            nc.tensor.matmul(
                                av_ps[:, 0:32], ets[d_][:],
                                st[d_]["vt"][:, i, g * 32:(g + 1) * 32],
                                start=True, stop=False)
                            nc.tensor.matmul(av_ps[:, 32:33], ets[d_][:],
                                             ones_s[:], start=False,
                                             stop=True)
                            avs[d_] = av_ps
                        xrns = {}
                        for d_ in range(2):
                            rcp = pcr.tile([128, 1], F32, tag="rcp",
                                           name=f"rcp{d_}", bufs=4)
                            nc.vector.reciprocal(rcp[:], avs[d_][:, 32:33])
                            xrn = pcr.tile([128, 32], BF, tag="xrn",
                                           name=f"xrn{d_}", bufs=4)
                            nc.vector.tensor_scalar(
                                xrn[:], avs[d_][:, 0:32], rcp[:], None,
                                ALU.mult)
                            xrns[d_] = xrn
                        for d_ in range(2):
                            nc.tensor.transpose(
                                asm_ps[d_][gg * 32:(gg + 1) * 32, :],
                                xrns[d_][:], idb_s[:],
                                tile_position=(0, gg * 32))
                    for d_ in range(2):
                        nc.scalar.activation(
                            st[d_]["xpre"][th][:, i * 128:(i + 1) * 128],
                            asm_ps[d_][:], AF.Relu)

            for d_ in range(2):
                wproj_d = wrow_s if d_ == 0 else wcol_s
                bcol = B_ROW if d_ == 0 else B_COL
                for mt in range(2):
                    ps = pcmm.tile([128, BL], F32, tag="cmm", bufs=2)
                    for kh in range(2):
                        nc.tensor.matmul(
                            ps[:],
                            wproj_d[:, (kh * 2 + mt) * 128:
                                    (kh * 2 + mt) * 128 + 128],
                            st[d_]["xpre"][kh][:], start=(kh == 0),
                            stop=(kh == 1))
                    nc.scalar.activation(
                        xproj[(d_, mt)][:], ps[:], AF.Identity,
                        bias=bia_s[:, bcol + mt:bcol + mt + 1])

          # =========================================================
          # Mega-loop: qkv[b] -> DW[b-1] -> PW[b-1] -> xx -> proj -> out
          # =========================================================
          with (
              tc.tile_pool(name="pf", bufs=1) as pf,
              tc.tile_pool(name="pfr", bufs=3) as pfr,
              tc.tile_pool(name="pbm", bufs=4, space="PSUM") as pbm,
              tc.tile_pool(name="pfd", bufs=3, space="PSUM") as pfd,
              tc.tile_pool(name="pfw", bufs=2, space="PSUM") as pfw,
          ):
            dwd_s = pf.tile([128, 36 * 128], BF)
            wpw_s = pf.tile([128, 4 * 256], BF)
            nc.sync.dma_start(dwd_s[:], dwd[:])
            nc.sync.dma_start(wpw_s[:], wpw[:])
            v_sb = [pf.tile([128, PSZ], BF, tag=f"v{h}", name=f"v{h}")
                    for h in range(2)]
            for t_ in v_sb:
                nc.gpsimd.memset(_ap(t_, 0, [[129 * PST, 2], [1, PST]]), 0.0)
                nc.gpsimd.memset(
                    _ap(t_, PST, [[PST, 128], [130, 2], [1, 2]]), 0.0)
            # q/k rings: 3 slots of 8 rows (2 halo + 4 data + 2 halo)
            qring = [pf.tile([128, SST], BF, name=f"qr{i}") for i in range(3)]
            kring = [pf.tile([128, SST], BF, name=f"kr{i}") for i in range(3)]
            for t_ in qring + kring:
                nc.gpsimd.memset(t_[:], 0.0)

            cbq = []
            def fetch_cb(bb):
                sl2 = slice(bb * BL, (bb + 1) * BL)
                t2 = [pfr.tile([128, BL], BF, tag=f"cbi{h}",
                               name=f"cbi{h}", bufs=5) for h in range(2)]
                nc.sync.dma_start(t2[0][:], cb_dram[0, :, sl2])
                nc.sync.dma_start(t2[1][:], cb_dram[1, :, sl2])
                cbq.append(t2)
            fetch_cb(0)
            fetch_cb(1)
            fetch_cb(2)
            for b in range(33):
              if b < 32:
                sl = slice(b * BL, (b + 1) * BL)
                if b + 3 < 32:
                    fetch_cb(b + 3)
                cbi = cbq[b]
                # q/k/v block b: 2 matmuls K=128 over cb halves + 1 K=16 (cf)
                for m, (wt, mt) in enumerate([(wqb_s, 0), (wkb_s, 0),
                                              (wvb_s, 0), (wvb_s, 1)]):
                    nt = 2 if wt is wvb_s else 1
                    ps = pbm.tile([128, BL], F32, tag="bmm",
                                  name=f"qps{m}")
                    for kh in range(2):
                        nc.tensor.matmul(
                            ps[:],
                            wt[:, (kh * nt + mt) * 128:
                               (kh * nt + mt) * 128 + 128],
                            cbi[kh][:], start=(kh == 0), stop=False)
                    nc.tensor.matmul(ps[:], aq_s[:, m * 128:(m + 1) * 128],
                                     cf[:, sl], start=False, stop=True)
                    bc = (B_Q, B_K, B_V, B_V + 1)[m]
                    if m < 2:
                        ring = qring if m == 0 else kring
                        pdst = _ap(ring[b % 3], 2 * PST + 2,
                                   [[PST, 4], [1, 128]])
                    else:
                        pdst = _ap(v_sb[m - 2], (4 * b + 1) * PST + 2,
                                   [[PST, 4], [1, 128]])
                    if m % 2 == 0:
                        nc.scalar.activation(
                            pdst, ps[:], AF.Identity,
                            bias=bia_s[:, bc:bc + 1])
                    else:
                        nc.vector.tensor_scalar(
                            pdst, ps[:], bia_s[:, bc:bc + 1],
                            None, ALU.add)
                # halo copies: slot b rows 0..1 <- slot b-1 rows 4..5;
                #              slot b-1 rows 6..7 <- slot b rows 2..3
                for ring in (qring, kring):
                    if b > 0:
                        nc.gpsimd.tensor_copy(
                            _ap(ring[b % 3], 0, [[1, 2 * PST]]),
                            _ap(ring[(b - 1) % 3], 4 * PST, [[1, 2 * PST]]))
                        nc.gpsimd.tensor_copy(
                            _ap(ring[(b - 1) % 3], 6 * PST, [[1, 2 * PST]]),
                            _ap(ring[b % 3], 2 * PST, [[1, 2 * PST]]))
                    else:
                        nc.gpsimd.memset(
                            _ap(ring[0], 0, [[1, 2 * PST]]), 0.0)
              if b == 0:
                  continue
              bp = b - 1          # tail block
              if bp == 31:
                  for ring in (qring, kring):
                      nc.gpsimd.memset(
                          _ap(ring[bp % 3], 6 * PST, [[1, 2 * PST]]), 0.0)

              # --- DW for block bp: 2-row groups j=0,1 ---
              dwg = [pfr.tile([128, 4 * 256], BF, tag=f"dwg{j}",
                              name=f"dwg{j}", bufs=3) for j in range(2)]
              for t in range(4):
                  dps = [pfd.tile([128, 260], F32, tag="dw",
                                  name=f"dps{j}", bufs=2) for j in range(2)]
                  for t9 in range(9):
                      ky, kx = divmod(t9, 3)
                      for j in range(2):
                          if t < 2:
                              src = (qring, kring)[t][bp % 3]
                              off = (1 + 2 * j + ky) * PST + kx + 1
                          else:
                              src = v_sb[t - 2]
                              off = (4 * bp + 2 * j + ky) * PST + kx + 1
                          nc.tensor.matmul(
                              _ap(dps[j], 0, [[1, 260]]),
                              dwd_s[:, (t * 9 + t9) * 128:
                                    (t * 9 + t9) * 128 + 128],
                              _ap(src, off, [[1, 260]]),
                              start=(t9 == 0), stop=(t9 == 8))
                  for j in range(2):
                      src = _ap(dps[j], 0, [[PST, 2], [1, 128]])
                      dst = _ap(dwg[j], t * 256, [[1, 256]])
                      if t % 2 == 0:
                          nc.scalar.activation(
                              dst, src, AF.Relu,
                              bias=bia_s[:, B_DW + t:B_DW + t + 1])
                      else:
                          nc.vector.tensor_scalar(
                              dst, src, bia_s[:, B_DW + t:B_DW + t + 1],
                              0.0, ALU.add, ALU.max)

              # --- PW: qo = w_pw @ dwg + b_pw ---
              pws = [pfw.tile([128, 2, 256], F32, tag="pw",
                              name=f"pws{j}") for j in range(2)]
              for mt in range(2):
                  for kt in range(4):
                      for j in range(2):
                          nc.tensor.matmul(
                              pws[j][:, mt, :],
                              wpw_s[:, kt * 256 + mt * 128:
                                    kt * 256 + mt * 128 + 128],
                              dwg[j][:, kt * 256:(kt + 1) * 256],
                              start=(kt == 0), stop=(kt == 3))
              qo_blk = [pfr.tile([128, BL], BF, tag=f"qo{mt}",
                                 name=f"qo{mt}", bufs=3) for mt in range(2)]
              for mt in range(2):
                  for j in range(2):
                      nc.scalar.activation(
                          qo_blk[mt][:, j * 256:(j + 1) * 256],
                          pws[j][:, mt, :], AF.Identity,
                          bias=bia_s[:, B_PW + mt:B_PW + mt + 1])

              # --- xx = relu(v + bcast(xrow) + bcast(xcol)) ---
              xxr = []
              for half in range(2):
                  xx = pfr.tile([128, BL], BF, tag=f"xx{half}",
                                name=f"xx{half}", bufs=3)
                  rap = _ap(xproj[(0, half)], bp * 16, [[1, 16], [0, 32]])
                  cap = _ap(xproj[(1, half)], (bp // 2) * 32,
                            [[0, 4], [0, 4], [1, 32]])
                  nc.vector.tensor_tensor(xx[:], rap, cap, ALU.add)
                  vap = _ap(v_sb[half], (4 * bp + 1) * PST + 2,
                            [[PST, 4], [1, 128]])
                  nc.vector.tensor_tensor(xx[:], xx[:], vap, ALU.add)
                  nc.vector.tensor_scalar(xx[:], xx[:], 0.0, None,
                                          ALU.max)
                  xxr.append(xx)

              # --- proj + hsig + multiply qo, DMA out ---
              for mt in range(2):
                  ps = pbm.tile([128, BL], F32, tag="bmm", name="prj")
                  for kh in range(2):
                      nc.tensor.matmul(
                          ps[:],
                          wproj_s[:, (kh * 2 + mt) * 128:
                                  (kh * 2 + mt) * 128 + 128],
                          xxr[kh][:], start=(kh == 0), stop=(kh == 1))
                  hs = pfr.tile([128, BL], BF, tag="hs", bufs=2)
                  nc.scalar.activation(
                      hs[:], ps[:], AF.Relu,
                      bias=bia_s[:, B_PROJ3 + mt:B_PROJ3 + mt + 1])
                  att_t = pfr.tile([128, BL], BF, tag="att", bufs=2)
                  nc.vector.tensor_scalar(att_t[:], hs[:], 6.0,
                                          1.0 / 6.0, ALU.min, ALU.mult)
                  ob = pfr.tile([128, BL], BF, tag="ob", bufs=2)
                  nc.vector.tensor_tensor(ob[:], att_t[:], qo_blk[mt][:],
                                          ALU.mult)
                  slp = slice(bp * BL, (bp + 1) * BL)
                  nc.gpsimd.dma_start(out[mt * 128:(mt + 1) * 128, slp],
                                      ob[:])

    nc.compile()
    return nc


def _interp_matrix():
    s, n = 16, 128
    src = np.clip((np.arange(n) + 0.5) * (s / n) - 0.5, 0.0, s - 1.0)
    i0 = np.floor(src).astype(np.int64)
    i1 = np.minimum(i0 + 1, s - 1)
    w = src - i0
    M = np.zeros((s, n), np.float64)
    np.add.at(M, (i0, np.arange(n)), 1.0 - w)
    np.add.at(M, (i1, np.arange(n)), w)
    return M


def _bf(x):
    return np.ascontiguousarray(np.asarray(x, np.float32).astype(
        ml_dtypes.bfloat16))


def prep_consts(inputs):
    """Host-side layout prep of all weight tensors (shared across cores)."""
    f = {k: np.asarray(v, np.float32) for k, v in inputs.items()}

    w3 = f["w_ccam_b"]                      # [256, 128, 3, 3]
    w3t = np.zeros((128, 9 * 256), np.float32)
    for ky in range(3):
        for kx in range(3):
            t9 = ky * 3 + kx
            w3t[:, t9 * 256:(t9 + 1) * 256] = w3[:, :, ky, kx].T
    wenc = np.zeros((128, 32), np.float32)  # w_enc [16, 256]
    for half in range(2):
        wenc[:, half * 16:(half + 1) * 16] = \
            f["w_enc"][:, half * 128:(half + 1) * 128].T

    def pack_lhsT(wm, nt):
        # wm [out, in]; returns [128, 2*nt*128]: [ci, (kh*nt+mt)*128+co]
        r = np.zeros((128, 2 * nt * 128), np.float32)
        for kh in range(2):
            for mt in range(nt):
                r[:, (kh * nt + mt) * 128:(kh * nt + mt) * 128 + 128] = \
                    wm[mt * 128:(mt + 1) * 128,
                       kh * 128:(kh + 1) * 128].T
        return r

    wq_p = pack_lhsT(f["w_q"], 1)
    wk_p = pack_lhsT(f["w_k"], 1)
    wv_p = pack_lhsT(f["w_v"], 2)
    wrow_p = pack_lhsT(f["w_row"], 2)
    wcol_p = pack_lhsT(f["w_col"], 2)
    wproj_p = pack_lhsT(f["w_proj"], 2)

    wpw_p = np.zeros((128, 4 * 256), np.float32)   # w_pw [256, 512]
    for kt in range(4):
        for mt in range(2):
            wpw_p[:, kt * 256 + mt * 128:kt * 256 + mt * 128 + 128] = \
                f["w_pw"][mt * 128:(mt + 1) * 128,
                          kt * 128:(kt + 1) * 128].T

    dwdg = np.zeros((128, 36 * 128), np.float32)   # w_dw [512,1,3,3]
    ii = np.arange(128)
    for t in range(4):
        for tap9 in range(9):
            ky, kx = divmod(tap9, 3)
            dwdg[ii, (t * 9 + tap9) * 128 + ii] = \
                f["w_dw"][t * 128 + ii, 0, ky, kx]

    post_p = np.zeros((16, 4 * 512), np.float32)
    for pidx, nm in enumerate(["pos_rowq", "pos_rowk", "pos_colq", "pos_colk"]):
        p = f[nm]                                   # [4, 128, 16]
        for i in range(4):
            post_p[:, (pidx * 4 + i) * 128:(pidx * 4 + i) * 128 + 128] = \
                p[i].T                              # [16, 128]

    biases = np.zeros((128, 20), np.float32)
    biases[:, B_CCAM + 0] = f["b_ccam_b"][:128]
    biases[:, B_CCAM + 1] = f["b_ccam_b"][128:]
    biases[:16, B_ENC] = f["b_enc"]
    biases[:, B_Q] = f["b_q"]
    biases[:, B_K] = f["b_k"]
    biases[:, B_V + 0] = f["b_v"][:128]
    biases[:, B_V + 1] = f["b_v"][128:]
    for t in range(4):
        biases[:, B_DW + t] = f["b_dw"][t * 128:(t + 1) * 128]
    biases[:, B_PW + 0] = f["b_pw"][:128]
    biases[:, B_PW + 1] = f["b_pw"][128:]
    biases[:, B_ROW + 0] = f["b_row"][:128]
    biases[:, B_ROW + 1] = f["b_row"][128:]
    biases[:, B_COL + 0] = f["b_col"][:128]
    biases[:, B_COL + 1] = f["b_col"][128:]
    biases[:, B_PROJ3 + 0] = f["b_proj"][:128] + 3.0
    biases[:, B_PROJ3 + 1] = f["b_proj"][128:] + 3.0

    return {
        "w3t": _bf(w3t), "wenc": _bf(wenc),
        "wq": _bf(wq_p), "wk": _bf(wk_p), "wv": _bf(wv_p),
        "wqs": _bf(wq_p / 32.0), "wks": _bf(wk_p / 32.0),
        "wvs": _bf(wv_p / 32.0),
        "dwd": _bf(dwdg), "wpw": _bf(wpw_p),
        "wrow": _bf(wrow_p), "wcol": _bf(wcol_p), "wproj": _bf(wproj_p),
        "post": _bf(post_p), "interpm": _bf(_interp_matrix()),
        "identb": _bf(np.eye(128)),
        "identf": np.eye(128, dtype=np.float32),
        "onesb": _bf(np.ones((128, 1))),
        "biases": np.ascontiguousarray(biases),
    }


def kernel(**inputs) -> np.ndarray:
    x = np.asarray(inputs["x"], np.float32)          # [8, 128, 128, 128]
    scale = float(np.asarray(inputs["scale_ccam"]).reshape(-1)[0])

    key = round(scale, 9)
    if key not in _CACHE:
        _CACHE[key] = build_graph(scale)
    nc = _CACHE[key]

    consts = prep_consts(inputs)
    in_maps = []
    for core in range(8):
        m = dict(consts)
        m["xb"] = np.ascontiguousarray(x[core].reshape(128, N))
        in_maps.append(m)

    res = run_bass_kernel_spmd(nc, in_maps, core_ids=list(range(8)))
    outs = [res.results[i]["out"].reshape(256, 128, 128) for i in range(8)]
    return np.stack(outs).astype(np.float32)


if __name__ == "__main__":
    rng = np.random.default_rng(0)
    demo = {"x": rng.standard_norm_((8, 128, 128, 128))} if False else None
    print("kernel module OK")
